# revision 19
# baseline (speedup 1.0000x reference)
"""Bass/Tile kernel for a 4-layer dense transformer (prefill) on 8 TRN2 cores.

Parallelization: 2-way data parallel (batch) x 4-way tensor parallel.
Groups: cores [0,1,2,3] handle batch 0, [4,5,6,7] batch 1.
Within a group (rank r = core % 4):
  - attention: heads r*4..r*4+3  (feature cols r*256..(r+1)*256)
  - MLP: hidden cols r*1024..(r+1)*1024
  - vocab: cols r*8000..(r+1)*8000 of head_w
Activations are kept TRANSPOSED on device: [feature(partition), token(free)].
Residual stream x is fp32; matmul inputs are bf16 (fp32 PSUM accumulation).
Per layer: AllGather(attn-out bf16), AllGather(attn-delta fp32),
AllGather(mlp-hidden bf16), AllGather(mlp-delta fp32).
Final logits are computed in natural [token, vocab] layout and written out
per-core as [1024, 8000]; the host concatenates.
"""

import sys
import types

import numpy as np


def _install_ntff_shim():
    """Register the NTFF profiling hook that trn_boot skipped (the image's
    antenv package lacks the axon_hooks submodule)."""
    if "antenv.axon_hooks" in sys.modules:
        return
    try:
        import trn_agent_boot.trn_boot as tb
        hook = tb._ntff_profile_via_ctypes("/opt/axon/libaxon_pjrt.so")
    except Exception:
        hook = None
    mod = types.ModuleType("antenv.axon_hooks")
    _h = [hook]
    mod.get_axon_ntff_profile_hook = lambda: _h[0]
    mod.set_axon_ntff_profile_hook = lambda h: _h.__setitem__(0, h)
    sys.modules["antenv.axon_hooks"] = mod
    try:
        import antenv
        antenv.axon_hooks = mod
    except Exception:
        pass


_install_ntff_shim()

import ml_dtypes
import concourse.bass as bass
import concourse.mybir as mybir
import concourse.tile as tile
from concourse import bacc
from concourse.bass_utils import run_bass_kernel_spmd

BF = mybir.dt.bfloat16
F32 = mybir.dt.float32
AL = mybir.AluOpType
AF = mybir.ActivationFunctionType

# Model sizes (full problem, hardcoded per contract).
CFG = dict(
    B=2, S=1024, V=32000, D=1024, H=16, L=4, EPS=1e-5,
    TP=4,            # tensor-parallel width (group size)
    gelu_sim=False,  # CoreSim lacks Gelu; use sigmoid-based stand-in
)

N_CORES = 8
GROUPS = [[0, 1, 2, 3], [4, 5, 6, 7]]


def build_program(cfg=None):
    """Build the SPMD Bass program (identical on all 8 cores)."""
    c = dict(CFG)
    if cfg:
        c.update(cfg)
    B, S, V, D, H, L = c["B"], c["S"], c["V"], c["D"], c["H"], c["L"]
    EPS, TP = c["EPS"], c["TP"]
    T = S                    # tokens per group (one batch element)
    DK = D // H              # head dim (64)
    HL = H // TP             # heads per core (4)
    DSH = D // TP            # attention/delta feature shard (256)
    DF = 4 * D
    DFS = DF // TP           # mlp hidden shard (1024)
    VSH = V // TP            # vocab shard (8000)
    KT = D // 128            # feature k-tiles (8)
    KTF = DF // 128          # mlp k-tiles (32)
    NCH = max(1, T // 512)   # token chunks of <=512
    TCH = min(512, T)        # token chunk size
    MSH = DSH // 128         # m-tiles of a DSH-wide output (2)
    TKT = T // 128           # key-token tiles (8)
    VCH = 500                # vocab chunk
    NV = VSH // VCH          # vocab n-chunks (16)
    TT = T // 128            # token tiles (8)
    assert T % 128 == 0 and D % 128 == 0 and DSH % 128 == 0
    assert VSH % NV == 0 and VCH <= 512

    groups = [[g * TP + r for r in range(TP)] for g in range(N_CORES // TP)]

    nc = bacc.Bacc("TRN2", target_bir_lowering=False, debug=False,
                   num_devices=N_CORES)

    # ---- DRAM parameters (per-core shards fed via in_maps) ----
    xT0 = nc.dram_tensor("xT0", [D, T], F32, kind="ExternalInput")
    xq0 = nc.dram_tensor("xq0", [D, T // TP], F32, kind="ExternalInput")
    wq = nc.dram_tensor("wq", [L, 128, KT, DSH], BF, kind="ExternalInput")
    wk = nc.dram_tensor("wk", [L, 128, KT, DSH], BF, kind="ExternalInput")
    wv = nc.dram_tensor("wv", [L, 128, KT, DSH], BF, kind="ExternalInput")
    wo = nc.dram_tensor("wo", [L, 128, DSH // 128, D], BF, kind="ExternalInput")
    w1 = nc.dram_tensor("w1", [L, 128, KTF, KT, 128], BF, kind="ExternalInput")
    w2 = nc.dram_tensor("w2", [L, 128, KT, KTF, 128], BF, kind="ExternalInput")
    b1 = nc.dram_tensor("b1", [L, DF], F32, kind="ExternalInput")
    b2 = nc.dram_tensor("b2", [L, D], F32, kind="ExternalInput")
    g1 = nc.dram_tensor("g1", [L, D], F32, kind="ExternalInput")
    be1 = nc.dram_tensor("be1", [L, D], F32, kind="ExternalInput")
    g2 = nc.dram_tensor("g2", [L, D], F32, kind="ExternalInput")
    be2 = nc.dram_tensor("be2", [L, D], F32, kind="ExternalInput")
    gf = nc.dram_tensor("gf", [1, D], F32, kind="ExternalInput")
    bef = nc.dram_tensor("bef", [1, D], F32, kind="ExternalInput")
    hw = nc.dram_tensor("hw", [NV, 128, KT, VCH], BF, kind="ExternalInput")
    logits = nc.dram_tensor("logits", [T, VSH], F32, kind="ExternalOutput")

    with tile.TileContext(nc) as tc:
        _build_tc(nc, tc, locals())
    nc.compile()
    return nc


def _build_tc(nc, tc, v):
    """Emit the tile program. `v` is the name->value dict from build_program."""
    (B, T, D, L, EPS, TP, DK, HL, DSH, DF, DFS, VSH, KT, KTF, NCH, TCH,
     MSH, TKT, NV, VCH, TT, groups) = (
        v["B"], v["T"], v["D"], v["L"], v["EPS"], v["TP"], v["DK"], v["HL"],
        v["DSH"], v["DF"], v["DFS"], v["VSH"], v["KT"], v["KTF"], v["NCH"],
        v["TCH"], v["MSH"], v["TKT"], v["NV"], v["VCH"], v["TT"], v["groups"])
    xT0, wq, wk, wv, wo, w1, w2 = (v["xT0"], v["wq"], v["wk"], v["wv"],
                                   v["wo"], v["w1"], v["w2"])
    b1d, b2d, g1d, be1d, g2d, be2d, gfd, befd = (
        v["b1"], v["b2"], v["g1"], v["be1"], v["g2"], v["be2"], v["gf"],
        v["bef"])
    hwd, logits = v["hw"], v["logits"]

    import contextlib
    ctx = contextlib.ExitStack()

    # ---------------- pools ----------------
    sing = ctx.enter_context(tc.tile_pool(name="sing", bufs=1))
    wts = ctx.enter_context(tc.tile_pool(name="wts", bufs=1))
    w1s = ctx.enter_context(tc.tile_pool(name="w1s", bufs=2))
    hwp = ctx.enter_context(tc.tile_pool(name="hwp", bufs=2))
    hp = ctx.enter_context(tc.tile_pool(name="hp", bufs=1))
    hq = ctx.enter_context(tc.tile_pool(name="hq", bufs=1))
    qkp = ctx.enter_context(tc.tile_pool(name="qkp", bufs=1))
    scr = ctx.enter_context(tc.tile_pool(name="scr", bufs=2))
    expp = ctx.enter_context(tc.tile_pool(name="expp", bufs=8))
    otp = ctx.enter_context(tc.tile_pool(name="otp", bufs=1))
    agf = ctx.enter_context(tc.tile_pool(name="agf", bufs=2))   # AG reads
    d8 = ctx.enter_context(tc.tile_pool(name="d8", bufs=1))     # attn delta
    dfp = ctx.enter_context(tc.tile_pool(name="dfp", bufs=1))   # RS-out quarter
    uqp = ctx.enter_context(tc.tile_pool(name="uqp", bufs=1))   # mlp hidden q
    lgp = ctx.enter_context(tc.tile_pool(name="lgp", bufs=2))
    tiny = ctx.enter_context(tc.tile_pool(name="tiny", bufs=2))
    rows1 = ctx.enter_context(tc.tile_pool(name="rows1", bufs=1))
    rows3 = ctx.enter_context(tc.tile_pool(name="rows3", bufs=3))
    rows2 = ctx.enter_context(tc.tile_pool(name="rows2", bufs=1))
    bcp = ctx.enter_context(tc.tile_pool(name="bcp", bufs=1))
    rbp = ctx.enter_context(tc.tile_pool(name="rbp", bufs=2))
    psmm = ctx.enter_context(tc.tile_pool(name="psmm", bufs=4, space="PSUM"))
    psaux = ctx.enter_context(tc.tile_pool(name="psaux", bufs=2, space="PSUM"))
    psst = ctx.enter_context(tc.tile_pool(name="psst", bufs=2, space="PSUM"))
    dram = ctx.enter_context(tc.tile_pool(name="dram", bufs=1, space="DRAM"))

    # ---------------- constants ----------------
    ones_col = sing.tile([128, 1], BF, name="ones_col")
    nc.vector.memset(ones_col, 1.0)
    ones_row = sing.tile([1, 128], BF, name="ones_row")
    nc.vector.memset(ones_row, 1.0)
    ones_row_f = sing.tile([1, 128], F32, name="ones_row_f")
    nc.vector.memset(ones_row_f, 1.0)
    eps_ap = sing.tile([1, 1], F32, name="eps_ap")
    nc.vector.memset(eps_ap, EPS)

    # ---------------- residual stream ----------------
    TQ = T // TP                 # local MLP tokens per rank (256)
    TQB = TQ // NCH              # per token-half block (128)
    x = [sing.tile([128, T], F32, name=f"x{k}") for k in range(KT)]
    xq = [sing.tile([128, TQ], F32, name=f"xq{k}") for k in range(KT)]
    for k in range(KT):
        nc.sync.dma_start(out=x[k], in_=xT0[k * 128:(k + 1) * 128, :])
        nc.sync.dma_start(out=xq[k], in_=v["xq0"][k * 128:(k + 1) * 128, :])

    # ---------------- layernorm ----------------
    def layernorm(x_tiles, grow_dram, brow_dram, name, ncols=None, nch=None,
                  hpool=None, htag="h"):
        """LN over the feature (partition) axis of transposed activations.
        Chunk-outer so chunk 0 proceeds while chunk 1's inputs are still
        being gathered. Returns bf16 tiles h[kt] = LN(x)."""
        ncols = T if ncols is None else ncols
        nch = NCH if nch is None else nch
        hpool = hp if hpool is None else hpool
        tchl = ncols // nch
        # per-partition gamma/beta columns: [128, KT]
        gcol = tiny.tile([128, KT], F32, name=f"g_{name}")
        bcol = tiny.tile([128, KT], F32, name=f"b_{name}")
        nc.sync.dma_start(out=gcol, in_=grow_dram.rearrange("(k p) -> p k", p=128))
        nc.sync.dma_start(out=bcol, in_=brow_dram.rearrange("(k p) -> p k", p=128))

        h = [hpool.tile([128, ncols], BF, name=f"h_{name}_{k}",
                        tag=f"{htag}{k}")
             for k in range(KT)]
        for ch in range(nch):
            cs = slice(ch * tchl, (ch + 1) * tchl)
            # stats: PSUM tile holds sum at partition 0, sumsq at 32.
            ps_st = psst.tile([33, tchl], F32, name="ps_st", tag="ps_st")
            for k in range(KT):
                xbt = scr.tile([128, tchl], BF, name="xb", tag="xb")
                nc.vector.tensor_copy(xbt, x_tiles[k][:, cs])
                sqt = scr.tile([128, tchl], BF, name="sq", tag="sq")
                nc.scalar.square(sqt, xbt)
                nc.tensor.matmul(ps_st[0:1, :], ones_col, xbt,
                                 start=(k == 0), stop=(k == KT - 1))
                nc.tensor.matmul(ps_st[32:33, :], ones_col, sqt,
                                 start=(k == 0), stop=(k == KT - 1))
            # moments for this chunk
            st_sb = rows1.tile([1, 2 * tchl], F32, name=f"st_{name}", tag="st_sb")
            nc.vector.tensor_copy(st_sb[:, 0:tchl], ps_st[0:1, :])
            nc.vector.tensor_copy(st_sb[:, tchl:2 * tchl], ps_st[32:33, :])
            mom = rows1.tile([1, 2 * tchl], F32, name=f"mom_{name}", tag="mom")
            nc.scalar.mul(mom, st_sb, 1.0 / D)      # [mean | E[x^2]]
            mean = mom[:, 0:tchl]
            msq = mom[:, tchl:2 * tchl]
            m2 = rows3.tile([1, tchl], F32, name=f"m2_{name}", tag="row1k")
            nc.vector.tensor_mul(m2, mean, mean)
            var = rows3.tile([1, tchl], F32, name=f"var_{name}", tag="row1k")
            nc.vector.tensor_tensor(out=var, in0=msq, in1=m2, op=AL.subtract)
            sd = rows3.tile([1, tchl], F32, name=f"sd_{name}", tag="row1k")
            nc.scalar.activation(sd, var, AF.Sqrt, bias=eps_ap)
            rstd = rows3.tile([1, tchl], F32, name=f"rstd_{name}", tag="row1k")
            nc.vector.reciprocal(rstd, sd)
            nmr = rows3.tile([1, tchl], F32, name=f"nmr_{name}", tag="row1k")
            nc.vector.tensor_mul(nmr, mean, rstd)
            nc.scalar.mul(nmr, nmr, -1.0)           # -mean*rstd
            # broadcast to [128, tchl] via K=1 outer-product matmuls (fp32)
            rstdB = bcp.tile([128, tchl], F32, name="rstdB", tag="rstdB")
            nmB = bcp.tile([128, tchl], F32, name="nmB", tag="nmB")
            pb = psaux.tile([128, tchl], F32, name="pb", tag="aux")
            nc.tensor.matmul(pb, ones_row_f, rstd, start=True, stop=True)
            nc.scalar.copy(rstdB, pb)
            pb2 = psaux.tile([128, tchl], F32, name="pb2", tag="aux")
            nc.tensor.matmul(pb2, ones_row_f, nmr, start=True, stop=True)
            nc.scalar.copy(nmB, pb2)
            # apply: h = (x*rstdB + nmB)*g + b, output bf16
            for k in range(KT):
                t1 = scr.tile([128, tchl], BF, name="lnt", tag="lnt")
                nc.vector.tensor_mul(t1, x_tiles[k][:, cs], rstdB)
                t2 = scr.tile([128, tchl], BF, name="lnt2", tag="lnt2")
                nc.vector.tensor_tensor(out=t2, in0=t1, in1=nmB, op=AL.add)
                nc.vector.tensor_scalar(
                    out=h[k][:, cs], in0=t2, scalar1=gcol[:, k:k + 1],
                    scalar2=bcol[:, k:k + 1], op0=AL.mult, op1=AL.add)
        return h

    # ---------------- transformer layers ----------------
    for l in range(L):
        # -- weights for this layer --
        wqt = wts.tile([128, KT, DSH], BF, name="wqt", tag="wqt")
        wkt = wts.tile([128, KT, DSH], BF, name="wkt", tag="wkt")
        wvt = wts.tile([128, KT, DSH], BF, name="wvt", tag="wvt")
        for dst, srcw in ((wqt, wq), (wkt, wk), (wvt, wv)):
            nc.sync.dma_start(out=dst, in_=srcw[l])
        # row-sharded Wo: [DSH local head feats, D] -> [128, 2, D]
        wot = wts.tile([128, MSH, D], BF, name="wot", tag="wot")
        nc.sync.dma_start(out=wot, in_=wo[l])
        b1col = tiny.tile([128, KTF], F32, name="b1col", tag="b1col")
        nc.sync.dma_start(out=b1col, in_=b1d[l].rearrange("(k p) -> p k", p=128))
        b2col = tiny.tile([128, KT], F32, name="b2col", tag="b2col")
        nc.sync.dma_start(out=b2col, in_=b2d[l].rearrange("(k p) -> p k", p=128))

        # -- LN1 --
        h1 = layernorm(x, g1d[l], be1d[l], f"ln1_{l}")

        # -- QKV projections (chunk-outer so attention c0 starts early) --
        # qT/kT: [DSH, T] transposed; v: natural [T, DSH] + ones column
        qT = [qkp.tile([128, T], BF, name=f"qT{m}", tag=f"qT{m}")
              for m in range(MSH)]
        kTt = [qkp.tile([128, T], BF, name=f"kT{m}", tag=f"kT{m}")
               for m in range(MSH)]
        vt = qkp.tile([128, TKT, HL, DK + 1], BF, name="vt", tag="vt")
        nc.vector.memset(vt[:, :, :, DK:DK + 1], 1.0)
        for chn in range(NCH):
            cs = slice(chn * TCH, (chn + 1) * TCH)
            for wt, dst in ((wkt, kTt), (wqt, qT)):
                pq = {}
                for m in range(MSH):
                    pq[m] = psmm.tile([128, TCH], F32, name="ps", tag="mm")
                for k in range(KT):
                    for m in range(MSH):
                        nc.tensor.matmul(pq[m],
                                         wt[:, k, m * 128:(m + 1) * 128],
                                         h1[k][:, cs],
                                         start=(k == 0), stop=(k == KT - 1))
                for m in range(MSH):
                    nc.vector.tensor_copy(dst[m][:, cs], pq[m])
            for t in range(chn * (TCH // 128), (chn + 1) * (TCH // 128)):
                ps = psmm.tile([128, TCH], F32, name="psv", tag="mm")
                for k in range(KT):
                    nc.tensor.matmul(ps[:, 0:DSH],
                                     h1[k][:, t * 128:(t + 1) * 128],
                                     wvt[:, k, :],
                                     start=(k == 0), stop=(k == KT - 1))
                nc.vector.tensor_copy(
                    vt[:, t, :, 0:DK],
                    ps[:, 0:DSH].rearrange("p (h d) -> p h d", h=HL))

        # -- attention, chunk-outer; Wo row-shard -> token ReduceScatter --
        rs_in = [dram.tile([TP, D, TQB], BF, name=f"rs_in{l}_{c}")
                 for c in range(NCH)]
        rs_out = [dram.tile([D, TQB], BF, name=f"rs_out{l}_{c}")
                  for c in range(NCH)]
        for chn in range(NCH):
            cs = slice(chn * TCH, (chn + 1) * TCH)
            jmax = (chn + 1) * (TCH // 128)
            oT = [otp.tile([128, TCH], BF, name=f"oT{m}", tag=f"oT{m}")
                  for m in range(MSH)]
            for hh in range(HL):
                mt = (hh * DK) // 128
                po = (hh * DK) % 128
                q_h = qT[mt][po:po + DK, :]
                k_h = kTt[mt][po:po + DK, :]
                exps = []
                for j in range(jmax):
                    pss = psmm.tile([128, TCH], F32, name="pss", tag="mm")
                    nc.tensor.matmul(pss, k_h[:, j * 128:(j + 1) * 128],
                                     q_h[:, cs], start=True, stop=True)
                    et = expp.tile([128, TCH], BF, name="exp", tag="exp")
                    nc.scalar.activation(et, pss, AF.Exp, scale=0.125)
                    if j * 128 >= chn * TCH:
                        # diagonal block: zero where tk_global > tq_global
                        nc.gpsimd.affine_select(
                            out=et, in_=et, pattern=[[1, TCH]],
                            compare_op=AL.is_ge, fill=0.0,
                            base=chn * TCH - j * 128, channel_multiplier=-1)
                    exps.append(et)
                ps_o = psaux.tile([DK + 1, TCH], F32, name="ps_o", tag="aux")
                for j in range(jmax):
                    nc.tensor.matmul(ps_o, vt[:, j, hh, :], exps[j],
                                     start=(j == 0), stop=(j == jmax - 1))
                rec = rows2.tile([1, TCH], F32, name="rec", tag="rec")
                den = rows2.tile([1, TCH], F32, name="den", tag="den")
                nc.vector.tensor_copy(den, ps_o[DK:DK + 1, :])
                rsc = rows2.tile([1, TCH], F32, name="rsc", tag="rsc")
                nc.vector.reciprocal_approx_accurate(rec, den, rsc)
                recb = rows2.tile([1, TCH], BF, name="recb", tag="recb")
                nc.vector.tensor_copy(recb, rec)
                ps_r = psmm.tile([128, TCH], F32, name="ps_r", tag="mm")
                nc.tensor.matmul(ps_r[0:DK, :], ones_row[:, 0:DK], recb,
                                 start=True, stop=True)
                rb = rbp.tile([DK, TCH], BF, name="rb", tag="rb")
                nc.scalar.copy(rb, ps_r[0:DK, :])
                nc.vector.tensor_tensor(
                    out=oT[mt][po:po + DK, :], in0=ps_o[0:DK, :], in1=rb,
                    op=AL.mult)
            # Wo row-shard: d1_part[m] = Wo[local rows].T @ o_local, full D
            d1_sb = [d8.tile([128, TCH], BF, name=f"d1s{m}", tag=f"d1s{m}")
                     for m in range(KT)]
            for m in range(KT):
                psd = psmm.tile([128, TCH], F32, name="psd1", tag="mm")
                for kk in range(MSH):
                    nc.tensor.matmul(psd,
                                     wot[:, kk, m * 128:(m + 1) * 128],
                                     oT[kk],
                                     start=(kk == 0), stop=(kk == MSH - 1))
                nc.vector.tensor_copy(d1_sb[m], psd)
                for b in range(TCH // TQB):
                    nc.sync.dma_start(
                        out=rs_in[chn][b, m * 128:(m + 1) * 128, :],
                        in_=d1_sb[m][:, b * TQB:(b + 1) * TQB])
            nc.gpsimd.collective_compute(
                "ReduceScatter", AL.add, replica_groups=groups,
                ins=[rs_in[chn].opt()], outs=[rs_out[chn].opt()])

        # -- local token-quarter residual: xq += d1q (kept for delta AG) --
        dfq = [dfp.tile([128, TQ], BF, name=f"dfq{k}", tag=f"dfq{k}")
               for k in range(KT)]
        for c in range(NCH):
            qs = slice(c * TQB, (c + 1) * TQB)
            for k in range(KT):
                nc.sync.dma_start(out=dfq[k][:, qs],
                                  in_=rs_out[c][k * 128:(k + 1) * 128, :])
                nc.vector.tensor_tensor(out=xq[k][:, qs], in0=xq[k][:, qs],
                                        in1=dfq[k][:, qs], op=AL.add)

        # -- LN2 on local quarter + full-width MLP on 256 tokens --
        h2q = layernorm(xq, g2d[l], be2d[l], f"ln2_{l}", ncols=TQ, nch=NCH,
                        hpool=hq, htag="hq")
        uq = uqp.tile([128, KTF, TQ], BF, name="uq", tag="uq")
        for m in range(KTF):
            w1m = w1s.tile([128, KT, 128], BF, name="w1m", tag="w1m")
            nc.sync.dma_start(out=w1m, in_=w1[l][:, m])
            pu = psmm.tile([128, TQ], F32, name="psu", tag="mm")
            for k in range(KT):
                nc.tensor.matmul(pu, w1m[:, k, :], h2q[k],
                                 start=(k == 0), stop=(k == KT - 1))
            nc.scalar.activation(uq[:, m, :], pu, AF.Gelu,
                                 bias=b1col[:, m:m + 1])

        # -- W2 full-width on local tokens; delta = d1q + z + b2 --
        for m in range(KT):
            w2m = w1s.tile([128, KTF, 128], BF, name="w2m", tag="w2m")
            nc.sync.dma_start(out=w2m, in_=w2[l][:, m])
            psz = psmm.tile([128, TQ], F32, name="psz", tag="mm")
            for kk in range(KTF):
                nc.tensor.matmul(psz, w2m[:, kk, :], uq[:, kk, :],
                                 start=(kk == 0), stop=(kk == KTF - 1))
            nc.vector.scalar_tensor_tensor(
                out=xq[m], in0=psz, scalar=b2col[:, m:m + 1],
                in1=xq[m], op0=AL.add, op1=AL.add)
            nc.vector.scalar_tensor_tensor(
                out=dfq[m], in0=psz, scalar=b2col[:, m:m + 1],
                in1=dfq[m], op0=AL.add, op1=AL.add)

        # -- AllGather layer deltas per token half; update full x --
        ag_in = [dram.tile([D, TQB], BF, name=f"ag_in{l}_{c}")
                 for c in range(NCH)]
        ag_out = [dram.tile([TP, D, TQB], BF, name=f"ag_out{l}_{c}")
                  for c in range(NCH)]
        for c in range(NCH):
            qs = slice(c * TQB, (c + 1) * TQB)
            for k in range(KT):
                nc.sync.dma_start(out=ag_in[c][k * 128:(k + 1) * 128, :],
                                  in_=dfq[k][:, qs])
            nc.gpsimd.collective_compute(
                "AllGather", AL.bypass, replica_groups=groups,
                ins=[ag_in[c].opt()], outs=[ag_out[c].opt()])
        for c in range(NCH):
            for rr in range(TP):
                xf = agf.tile([128, KT, TQB], BF, name="xf", tag="agf")
                nc.sync.dma_start(
                    out=xf,
                    in_=ag_out[c][rr].rearrange("(k p) t -> p k t", p=128))
                tb = c * TCH + rr * TQB
                for k in range(KT):
                    nc.vector.tensor_tensor(
                        out=x[k][:, tb:tb + TQB], in0=x[k][:, tb:tb + TQB],
                        in1=xf[:, k, :], op=AL.add)

    # ---------------- final LN + logits ----------------
    hf = layernorm(x, gfd[0], befd[0], "lnf")
    for n in range(NV):
        hb = hwp.tile([128, KT, VCH], BF, name="hwb", tag="hwb")
        nc.sync.dma_start(out=hb, in_=hwd[n])
        for t in range(TT):
            ps = psmm.tile([128, TCH], F32, name="pslg", tag="mm")
            for k in range(KT):
                nc.tensor.matmul(ps[:, 0:VCH],
                                 hf[k][:, t * 128:(t + 1) * 128],
                                 hb[:, k, :],
                                 start=(k == 0), stop=(k == KT - 1))
            lg = lgp.tile([128, VCH], F32, name="lg", tag="lg")
            nc.vector.tensor_copy(lg, ps[:, 0:VCH])
            nc.sync.dma_start(
                out=logits[t * 128:(t + 1) * 128, n * VCH:(n + 1) * VCH],
                in_=lg)

    ctx.close()


# ---------------- host side ----------------

_PROG_CACHE = {}


def _get_program():
    if "nc" not in _PROG_CACHE:
        _PROG_CACHE["nc"] = build_program()
    return _PROG_CACHE["nc"]


def make_in_maps(input_ids, emb, Wq, Wk, Wv, Wo, W1, b1, W2, b2,
                 ln1_g, ln1_b, ln2_g, ln2_b, lnf_g, lnf_b, head_w):
    TP = CFG["TP"]
    D, V = CFG["D"], CFG["V"]
    DSH, DFS, VSH = D // TP, 4 * D // TP, V // TP
    bf = ml_dtypes.bfloat16
    in_maps = []
    S = CFG["S"]
    L = CFG["L"]
    TQB = S // TP // 2
    KT, KTF = D // 128, 4 * D // 128
    w1f = np.ascontiguousarray(
        np.asarray(W1).reshape(L, KT, 128, KTF, 128)
        .transpose(0, 2, 3, 1, 4)).astype(bf)
    w2f = np.ascontiguousarray(
        np.asarray(W2).reshape(L, KTF, 128, KT, 128)
        .transpose(0, 2, 3, 1, 4)).astype(bf)
    b1f = np.ascontiguousarray(b1).astype(np.float32)
    def _wtile(wfull):
        # [L, D, DSH] -> [L, 128, KT, DSH]
        a = np.asarray(wfull)
        return np.ascontiguousarray(
            a.reshape(L, KT, 128, a.shape[-1]).transpose(0, 2, 1, 3))
    for c in range(N_CORES):
        g, r = c // TP, c % TP
        x0 = np.asarray(emb)[np.asarray(input_ids)[g]]          # [S, D] f32
        x0T = np.ascontiguousarray(x0.T).astype(np.float32)
        xq0 = np.concatenate(
            [x0T[:, r * TQB:(r + 1) * TQB],
             x0T[:, S // 2 + r * TQB:S // 2 + (r + 1) * TQB]], axis=1)
        in_maps.append({
            "xT0": x0T,
            "xq0": np.ascontiguousarray(xq0).astype(np.float32),
            "wq": _wtile(Wq[:, :, r * DSH:(r + 1) * DSH]).astype(bf),
            "wk": _wtile(Wk[:, :, r * DSH:(r + 1) * DSH]).astype(bf),
            "wv": _wtile(Wv[:, :, r * DSH:(r + 1) * DSH]).astype(bf),
            "wo": np.ascontiguousarray(
                np.asarray(Wo)[:, r * DSH:(r + 1) * DSH, :]
                .reshape(L, DSH // 128, 128, D)
                .transpose(0, 2, 1, 3)).astype(bf),
            "w1": w1f,
            "w2": w2f,
            "b1": b1f,
            "b2": np.asarray(b2, dtype=np.float32),
            "g1": np.asarray(ln1_g, dtype=np.float32),
            "be1": np.asarray(ln1_b, dtype=np.float32),
            "g2": np.asarray(ln2_g, dtype=np.float32),
            "be2": np.asarray(ln2_b, dtype=np.float32),
            "gf": np.asarray(lnf_g, dtype=np.float32).reshape(1, -1),
            "bef": np.asarray(lnf_b, dtype=np.float32).reshape(1, -1),
            "hw": np.ascontiguousarray(
                np.asarray(head_w)[:, r * VSH:(r + 1) * VSH]
                .reshape(KT, 128, 16, 500).transpose(2, 1, 0, 3)).astype(bf),
        })
    return in_maps


def kernel(**inputs):
    B, S, V = CFG["B"], CFG["S"], CFG["V"]
    TP = CFG["TP"]
    VSH = V // TP
    nc = _get_program()
    in_maps = make_in_maps(**inputs)
    res = run_bass_kernel_spmd(nc, in_maps, list(range(N_CORES)), trace=False)
    out = np.empty((B, S, V), dtype=np.float32)
    for c in range(N_CORES):
        g, r = c // TP, c % TP
        out[g, :, r * VSH:(r + 1) * VSH] = res.results[c]["logits"]
    return out


def run_traced(**inputs):
    """Like kernel() but with NTFF tracing; returns (out, exec_time_ns)."""
    nc = _get_program()
    in_maps = make_in_maps(**inputs)
    res = run_bass_kernel_spmd(nc, in_maps, list(range(N_CORES)), trace=True)
    B, S, V = CFG["B"], CFG["S"], CFG["V"]
    TP = CFG["TP"]
    VSH = V // TP
    out = np.empty((B, S, V), dtype=np.float32)
    for c in range(N_CORES):
        g, r = c // TP, c % TP
        out[g, :, r * VSH:(r + 1) * VSH] = res.results[c]["logits"]
    return out, res.exec_time_ns



# revision 24
# speedup vs baseline: 1.0183x; 1.0183x over previous
"""Bass/Tile kernel for a 4-layer dense transformer (prefill) on 8 TRN2 cores.

Parallelization: 2-way data parallel (batch) x 4-way tensor parallel.
Groups: cores [0,1,2,3] handle batch 0, [4,5,6,7] batch 1.
Within a group (rank r = core % 4):
  - attention: heads r*4..r*4+3  (feature cols r*256..(r+1)*256)
  - MLP: hidden cols r*1024..(r+1)*1024
  - vocab: cols r*8000..(r+1)*8000 of head_w
Activations are kept TRANSPOSED on device: [feature(partition), token(free)].
Residual stream x is fp32; matmul inputs are bf16 (fp32 PSUM accumulation).
Per layer: AllGather(attn-out bf16), AllGather(attn-delta fp32),
AllGather(mlp-hidden bf16), AllGather(mlp-delta fp32).
Final logits are computed in natural [token, vocab] layout and written out
per-core as [1024, 8000]; the host concatenates.
"""

import sys
import types

import numpy as np


def _install_ntff_shim():
    """Register the NTFF profiling hook that trn_boot skipped (the image's
    antenv package lacks the axon_hooks submodule)."""
    if "antenv.axon_hooks" in sys.modules:
        return
    try:
        import trn_agent_boot.trn_boot as tb
        hook = tb._ntff_profile_via_ctypes("/opt/axon/libaxon_pjrt.so")
    except Exception:
        hook = None
    mod = types.ModuleType("antenv.axon_hooks")
    _h = [hook]
    mod.get_axon_ntff_profile_hook = lambda: _h[0]
    mod.set_axon_ntff_profile_hook = lambda h: _h.__setitem__(0, h)
    sys.modules["antenv.axon_hooks"] = mod
    try:
        import antenv
        antenv.axon_hooks = mod
    except Exception:
        pass


_install_ntff_shim()

import ml_dtypes
import concourse.bass as bass
import concourse.mybir as mybir
import concourse.tile as tile
from concourse import bacc
from concourse.bass_utils import run_bass_kernel_spmd

BF = mybir.dt.bfloat16
F32 = mybir.dt.float32
AL = mybir.AluOpType
AF = mybir.ActivationFunctionType

# Model sizes (full problem, hardcoded per contract).
CFG = dict(
    B=2, S=1024, V=32000, D=1024, H=16, L=4, EPS=1e-5,
    TP=4,            # tensor-parallel width (group size)
    gelu_sim=False,  # CoreSim lacks Gelu; use sigmoid-based stand-in
)

N_CORES = 8
GROUPS = [[0, 1, 2, 3], [4, 5, 6, 7]]


def build_program(cfg=None):
    """Build the SPMD Bass program (identical on all 8 cores)."""
    c = dict(CFG)
    if cfg:
        c.update(cfg)
    B, S, V, D, H, L = c["B"], c["S"], c["V"], c["D"], c["H"], c["L"]
    EPS, TP = c["EPS"], c["TP"]
    T = S                    # tokens per group (one batch element)
    DK = D // H              # head dim (64)
    HL = H // TP             # heads per core (4)
    DSH = D // TP            # attention/delta feature shard (256)
    DF = 4 * D
    DFS = DF // TP           # mlp hidden shard (1024)
    VSH = V // TP            # vocab shard (8000)
    KT = D // 128            # feature k-tiles (8)
    KTF = DF // 128          # mlp k-tiles (32)
    NCH = max(1, T // 512)   # token chunks of <=512
    TCH = min(512, T)        # token chunk size
    MSH = DSH // 128         # m-tiles of a DSH-wide output (2)
    TKT = T // 128           # key-token tiles (8)
    VCH = 500                # vocab chunk
    NV = VSH // VCH          # vocab n-chunks (16)
    TT = T // 128            # token tiles (8)
    assert T % 128 == 0 and D % 128 == 0 and DSH % 128 == 0
    assert VSH % NV == 0 and VCH <= 512

    groups = [[g * TP + r for r in range(TP)] for g in range(N_CORES // TP)]

    nc = bacc.Bacc("TRN2", target_bir_lowering=False, debug=False,
                   num_devices=N_CORES)

    # ---- DRAM parameters (per-core shards fed via in_maps) ----
    xT0 = nc.dram_tensor("xT0", [D, T], F32, kind="ExternalInput")
    xq0 = nc.dram_tensor("xq0", [D, T // TP], F32, kind="ExternalInput")
    wq = nc.dram_tensor("wq", [L, 128, KT, DSH], BF, kind="ExternalInput")
    wk = nc.dram_tensor("wk", [L, 128, KT, DSH], BF, kind="ExternalInput")
    wv = nc.dram_tensor("wv", [L, 128, KT, DSH], BF, kind="ExternalInput")
    wo = nc.dram_tensor("wo", [L, 128, DSH // 128, D], BF, kind="ExternalInput")
    w1 = nc.dram_tensor("w1", [L, 128, KTF, KT, 128], BF, kind="ExternalInput")
    w2 = nc.dram_tensor("w2", [L, 128, KT, KTF, 128], BF, kind="ExternalInput")
    b1 = nc.dram_tensor("b1", [L, DF], F32, kind="ExternalInput")
    b2 = nc.dram_tensor("b2", [L, D], F32, kind="ExternalInput")
    g1 = nc.dram_tensor("g1", [L, D], F32, kind="ExternalInput")
    be1 = nc.dram_tensor("be1", [L, D], F32, kind="ExternalInput")
    g2 = nc.dram_tensor("g2", [L, D], F32, kind="ExternalInput")
    be2 = nc.dram_tensor("be2", [L, D], F32, kind="ExternalInput")
    gf = nc.dram_tensor("gf", [1, D], F32, kind="ExternalInput")
    bef = nc.dram_tensor("bef", [1, D], F32, kind="ExternalInput")
    hw = nc.dram_tensor("hw", [NV, 128, KT, VCH], BF, kind="ExternalInput")
    sel2d = nc.dram_tensor("sel2d", [65, 128], F32, kind="ExternalInput")
    logits = nc.dram_tensor("logits", [T, VSH], F32, kind="ExternalOutput")

    with tile.TileContext(nc) as tc:
        _build_tc(nc, tc, locals())
    nc.compile()
    return nc


def _build_tc(nc, tc, v):
    """Emit the tile program. `v` is the name->value dict from build_program."""
    (B, T, D, L, EPS, TP, DK, HL, DSH, DF, DFS, VSH, KT, KTF, NCH, TCH,
     MSH, TKT, NV, VCH, TT, groups) = (
        v["B"], v["T"], v["D"], v["L"], v["EPS"], v["TP"], v["DK"], v["HL"],
        v["DSH"], v["DF"], v["DFS"], v["VSH"], v["KT"], v["KTF"], v["NCH"],
        v["TCH"], v["MSH"], v["TKT"], v["NV"], v["VCH"], v["TT"], v["groups"])
    xT0, wq, wk, wv, wo, w1, w2 = (v["xT0"], v["wq"], v["wk"], v["wv"],
                                   v["wo"], v["w1"], v["w2"])
    b1d, b2d, g1d, be1d, g2d, be2d, gfd, befd = (
        v["b1"], v["b2"], v["g1"], v["be1"], v["g2"], v["be2"], v["gf"],
        v["bef"])
    hwd, logits = v["hw"], v["logits"]

    import contextlib
    ctx = contextlib.ExitStack()

    # ---------------- pools ----------------
    sing = ctx.enter_context(tc.tile_pool(name="sing", bufs=1))
    wts = ctx.enter_context(tc.tile_pool(name="wts", bufs=1))
    w1s = ctx.enter_context(tc.tile_pool(name="w1s", bufs=2))
    hwp = ctx.enter_context(tc.tile_pool(name="hwp", bufs=2))
    hp = ctx.enter_context(tc.tile_pool(name="hp", bufs=1))
    hq = ctx.enter_context(tc.tile_pool(name="hq", bufs=1))
    qkp = ctx.enter_context(tc.tile_pool(name="qkp", bufs=1))
    scr = ctx.enter_context(tc.tile_pool(name="scr", bufs=2))
    expp = ctx.enter_context(tc.tile_pool(name="expp", bufs=8))
    otp = ctx.enter_context(tc.tile_pool(name="otp", bufs=1))
    agf = ctx.enter_context(tc.tile_pool(name="agf", bufs=2))   # AG reads
    d8 = ctx.enter_context(tc.tile_pool(name="d8", bufs=1))     # attn delta
    dfp = ctx.enter_context(tc.tile_pool(name="dfp", bufs=1))   # RS-out quarter
    uqp = ctx.enter_context(tc.tile_pool(name="uqp", bufs=1))   # mlp hidden q
    lgp = ctx.enter_context(tc.tile_pool(name="lgp", bufs=2))
    tiny = ctx.enter_context(tc.tile_pool(name="tiny", bufs=2))
    rows1 = ctx.enter_context(tc.tile_pool(name="rows1", bufs=1))
    rows3 = ctx.enter_context(tc.tile_pool(name="rows3", bufs=3))
    rows2 = ctx.enter_context(tc.tile_pool(name="rows2", bufs=1))
    bcp = ctx.enter_context(tc.tile_pool(name="bcp", bufs=1))
    rbp = ctx.enter_context(tc.tile_pool(name="rbp", bufs=2))
    psmm = ctx.enter_context(tc.tile_pool(name="psmm", bufs=4, space="PSUM"))
    psaux = ctx.enter_context(tc.tile_pool(name="psaux", bufs=3, space="PSUM"))
    psst = ctx.enter_context(tc.tile_pool(name="psst", bufs=1, space="PSUM"))
    dram = ctx.enter_context(tc.tile_pool(name="dram", bufs=1, space="DRAM"))

    # ---------------- constants ----------------
    ones_col = sing.tile([128, 1], BF, name="ones_col")
    nc.vector.memset(ones_col, 1.0)
    ones_row = sing.tile([1, 128], BF, name="ones_row")
    nc.vector.memset(ones_row, 1.0)
    ones_row_f = sing.tile([1, 128], F32, name="ones_row_f")
    nc.vector.memset(ones_row_f, 1.0)
    eps_ap = sing.tile([1, 1], F32, name="eps_ap")
    nc.vector.memset(eps_ap, EPS)
    sel2 = sing.tile([65, 128], F32, name="sel2")
    nc.sync.dma_start(out=sel2, in_=v["sel2d"][:, :])

    # ---------------- residual stream ----------------
    TQ = T // TP                 # local MLP tokens per rank (256)
    TQB = TQ // NCH              # per token-half block (128)
    x = [sing.tile([128, T], F32, name=f"x{k}") for k in range(KT)]
    xq = [sing.tile([128, TQ], F32, name=f"xq{k}") for k in range(KT)]
    for k in range(KT):
        nc.sync.dma_start(out=x[k], in_=xT0[k * 128:(k + 1) * 128, :])
        nc.sync.dma_start(out=xq[k], in_=v["xq0"][k * 128:(k + 1) * 128, :])

    # ---------------- layernorm ----------------
    def layernorm(x_tiles, grow_dram, brow_dram, name, ncols=None, nch=None,
                  hpool=None, htag="h"):
        """LN over the feature (partition) axis of transposed activations.
        Chunk-outer so chunk 0 proceeds while chunk 1's inputs are still
        being gathered. Returns bf16 tiles h[kt] = LN(x)."""
        ncols = T if ncols is None else ncols
        nch = NCH if nch is None else nch
        hpool = hp if hpool is None else hpool
        tchl = ncols // nch
        # per-partition gamma/beta columns: [128, KT]
        gcol = tiny.tile([128, KT], F32, name=f"g_{name}")
        bcol = tiny.tile([128, KT], F32, name=f"b_{name}")
        nc.sync.dma_start(out=gcol, in_=grow_dram.rearrange("(k p) -> p k", p=128))
        nc.sync.dma_start(out=bcol, in_=brow_dram.rearrange("(k p) -> p k", p=128))

        h = [hpool.tile([128, ncols], BF, name=f"h_{name}_{k}",
                        tag=f"{htag}{k}")
             for k in range(KT)]
        for ch in range(nch):
            cs = slice(ch * tchl, (ch + 1) * tchl)
            # stats: PSUM tile holds sum at partition 0, sumsq at 32.
            ps_st = psst.tile([33, tchl], F32, name="ps_st", tag="ps_st")
            for k in range(KT):
                xbt = scr.tile([128, tchl], BF, name="xb", tag="xb")
                nc.vector.tensor_copy(xbt, x_tiles[k][:, cs])
                sqt = scr.tile([128, tchl], BF, name="sq", tag="sq")
                nc.scalar.square(sqt, xbt)
                nc.tensor.matmul(ps_st[0:1, :], ones_col, xbt,
                                 start=(k == 0), stop=(k == KT - 1))
                nc.tensor.matmul(ps_st[32:33, :], ones_col, sqt,
                                 start=(k == 0), stop=(k == KT - 1))
            # moments for this chunk
            st_sb = rows1.tile([1, 2 * tchl], F32, name=f"st_{name}", tag="st_sb")
            nc.vector.tensor_copy(st_sb[:, 0:tchl], ps_st[0:1, :])
            nc.vector.tensor_copy(st_sb[:, tchl:2 * tchl], ps_st[32:33, :])
            mom = rows1.tile([1, 2 * tchl], F32, name=f"mom_{name}", tag="mom")
            nc.scalar.mul(mom, st_sb, 1.0 / D)      # [mean | E[x^2]]
            mean = mom[:, 0:tchl]
            msq = mom[:, tchl:2 * tchl]
            m2 = rows3.tile([1, tchl], F32, name=f"m2_{name}", tag="row1k")
            nc.vector.tensor_mul(m2, mean, mean)
            var = rows3.tile([1, tchl], F32, name=f"var_{name}", tag="row1k")
            nc.vector.tensor_tensor(out=var, in0=msq, in1=m2, op=AL.subtract)
            sd = rows3.tile([1, tchl], F32, name=f"sd_{name}", tag="row1k")
            nc.scalar.activation(sd, var, AF.Sqrt, bias=eps_ap)
            rstd = rows3.tile([1, tchl], F32, name=f"rstd_{name}", tag="row1k")
            nc.vector.reciprocal(rstd, sd)
            nmr = rows3.tile([1, tchl], F32, name=f"nmr_{name}", tag="row1k")
            nc.vector.tensor_mul(nmr, mean, rstd)
            nc.scalar.mul(nmr, nmr, -1.0)           # -mean*rstd
            # broadcast to [128, tchl] via K=1 outer-product matmuls (fp32)
            rstdB = bcp.tile([128, tchl], F32, name="rstdB", tag="rstdB")
            nmB = bcp.tile([128, tchl], F32, name="nmB", tag="nmB")
            pb = psaux.tile([128, tchl], F32, name="pb", tag="aux")
            nc.tensor.matmul(pb, ones_row_f, rstd, start=True, stop=True)
            nc.scalar.copy(rstdB, pb)
            pb2 = psaux.tile([128, tchl], F32, name="pb2", tag="aux")
            nc.tensor.matmul(pb2, ones_row_f, nmr, start=True, stop=True)
            nc.scalar.copy(nmB, pb2)
            # apply: h = (x*rstdB + nmB)*g + b, output bf16
            for k in range(KT):
                t1 = scr.tile([128, tchl], BF, name="lnt", tag="lnt")
                nc.vector.tensor_mul(t1, x_tiles[k][:, cs], rstdB)
                t2 = scr.tile([128, tchl], BF, name="lnt2", tag="lnt2")
                nc.vector.tensor_tensor(out=t2, in0=t1, in1=nmB, op=AL.add)
                nc.vector.tensor_scalar(
                    out=h[k][:, cs], in0=t2, scalar1=gcol[:, k:k + 1],
                    scalar2=bcol[:, k:k + 1], op0=AL.mult, op1=AL.add)
        return h

    # ---------------- transformer layers ----------------
    for l in range(L):
        # -- weights for this layer --
        wqt = wts.tile([128, KT, DSH], BF, name="wqt", tag="wqt")
        wkt = wts.tile([128, KT, DSH], BF, name="wkt", tag="wkt")
        wvt = wts.tile([128, KT, DSH], BF, name="wvt", tag="wvt")
        for dst, srcw in ((wqt, wq), (wkt, wk), (wvt, wv)):
            nc.sync.dma_start(out=dst, in_=srcw[l])
        # row-sharded Wo: [DSH local head feats, D] -> [128, 2, D]
        wot = wts.tile([128, MSH, D], BF, name="wot", tag="wot")
        nc.sync.dma_start(out=wot, in_=wo[l])
        b1col = tiny.tile([128, KTF], F32, name="b1col", tag="b1col")
        nc.sync.dma_start(out=b1col, in_=b1d[l].rearrange("(k p) -> p k", p=128))
        b2col = tiny.tile([128, KT], F32, name="b2col", tag="b2col")
        nc.sync.dma_start(out=b2col, in_=b2d[l].rearrange("(k p) -> p k", p=128))

        # -- LN1 --
        h1 = layernorm(x, g1d[l], be1d[l], f"ln1_{l}")

        # -- QKV projections (chunk-outer so attention c0 starts early) --
        # qT/kT: [DSH, T] transposed; v: natural [T, DSH] + ones column
        qT = [qkp.tile([128, T], BF, name=f"qT{m}", tag=f"qT{m}")
              for m in range(MSH)]
        kTt = [qkp.tile([128, T], BF, name=f"kT{m}", tag=f"kT{m}")
               for m in range(MSH)]
        vt = qkp.tile([128, TKT, HL, DK + 1], BF, name="vt", tag="vt")
        nc.vector.memset(vt[:, :, :, DK:DK + 1], 1.0)
        for chn in range(NCH):
            cs = slice(chn * TCH, (chn + 1) * TCH)
            for wt, dst in ((wkt, kTt), (wqt, qT)):
                pq = {}
                for m in range(MSH):
                    pq[m] = psmm.tile([128, TCH], F32, name="ps", tag="mm")
                for k in range(KT):
                    for m in range(MSH):
                        nc.tensor.matmul(pq[m],
                                         wt[:, k, m * 128:(m + 1) * 128],
                                         h1[k][:, cs],
                                         start=(k == 0), stop=(k == KT - 1))
                for m in range(MSH):
                    nc.vector.tensor_copy(dst[m][:, cs], pq[m])
            for t in range(chn * (TCH // 128), (chn + 1) * (TCH // 128)):
                ps = psmm.tile([128, TCH], F32, name="psv", tag="mm")
                for k in range(KT):
                    nc.tensor.matmul(ps[:, 0:DSH],
                                     h1[k][:, t * 128:(t + 1) * 128],
                                     wvt[:, k, :],
                                     start=(k == 0), stop=(k == KT - 1))
                nc.vector.tensor_copy(
                    vt[:, t, :, 0:DK],
                    ps[:, 0:DSH].rearrange("p (h d) -> p h d", h=HL))

        # -- attention, chunk-outer; Wo row-shard -> token ReduceScatter --
        rs_in = [dram.tile([TP, D, TQB], BF, name=f"rs_in{l}_{c}")
                 for c in range(NCH)]
        rs_out = [dram.tile([D, TQB], BF, name=f"rs_out{l}_{c}")
                  for c in range(NCH)]
        for chn in range(NCH):
            cs = slice(chn * TCH, (chn + 1) * TCH)
            jmax = (chn + 1) * (TCH // 128)
            oT = [otp.tile([128, TCH], BF, name=f"oT{m}", tag=f"oT{m}")
                  for m in range(MSH)]
            for hpi in range(HL // 2):
                # heads (2hp, 2hp+1) sit at partitions 0-63 / 64-127 of
                # m-tile hp: their K=64 score matmuls land in different PE
                # row-groups and run concurrently.
                mt = hpi
                ps_os = [psaux.tile([DK + 1, TCH], F32, name="ps_o",
                                    tag="aux") for _ in range(2)]
                for j in range(jmax):
                    ets = []
                    for sub in range(2):
                        po = sub * DK
                        pss = psmm.tile([128, TCH], F32, name="pss",
                                        tag="mm")
                        nc.tensor.matmul(
                            pss, kTt[mt][po:po + DK, j * 128:(j + 1) * 128],
                            qT[mt][po:po + DK, cs], start=True, stop=True)
                        et = expp.tile([128, TCH], BF, name="exp", tag="exp")
                        nc.scalar.activation(et, pss, AF.Exp, scale=0.125)
                        if j * 128 >= chn * TCH:
                            # zero where tk_global > tq_global
                            nc.gpsimd.affine_select(
                                out=et, in_=et, pattern=[[1, TCH]],
                                compare_op=AL.is_ge, fill=0.0,
                                base=chn * TCH - j * 128,
                                channel_multiplier=-1)
                        ets.append(et)
                    for sub in range(2):
                        nc.tensor.matmul(ps_os[sub], vt[:, j, 2 * hpi + sub, :],
                                         ets[sub],
                                         start=(j == 0), stop=(j == jmax - 1))
                den2 = rows2.tile([65, TCH], F32, name="den2", tag="den")
                nc.vector.memset(den2, 1.0)
                nc.vector.tensor_copy(den2[0:1, :], ps_os[0][DK:DK + 1, :])
                nc.vector.tensor_copy(den2[DK:DK + 1, :],
                                      ps_os[1][DK:DK + 1, :])
                rec2 = rows2.tile([65, TCH], F32, name="rec2", tag="rec")
                rsc2 = rows2.tile([65, TCH], F32, name="rsc2", tag="rsc")
                nc.vector.reciprocal_approx_accurate(rec2, den2, rsc2)
                ps_r = psmm.tile([128, TCH], F32, name="ps_r", tag="mm")
                nc.tensor.matmul(ps_r, sel2, rec2, start=True, stop=True)
                rb = rbp.tile([128, TCH], BF, name="rb", tag="rb")
                nc.scalar.copy(rb, ps_r)
                for sub in range(2):
                    po = sub * DK
                    nc.vector.tensor_tensor(
                        out=oT[mt][po:po + DK, :], in0=ps_os[sub][0:DK, :],
                        in1=rb[po:po + DK, :], op=AL.mult)
            # Wo row-shard: d1_part[m] = Wo[local rows].T @ o_local, full D
            d1_sb = [d8.tile([128, TCH], BF, name=f"d1s{m}", tag=f"d1s{m}")
                     for m in range(KT)]
            for m in range(KT):
                psd = psmm.tile([128, TCH], F32, name="psd1", tag="mm")
                for kk in range(MSH):
                    nc.tensor.matmul(psd,
                                     wot[:, kk, m * 128:(m + 1) * 128],
                                     oT[kk],
                                     start=(kk == 0), stop=(kk == MSH - 1))
                nc.vector.tensor_copy(d1_sb[m], psd)
                for b in range(TCH // TQB):
                    nc.sync.dma_start(
                        out=rs_in[chn][b, m * 128:(m + 1) * 128, :],
                        in_=d1_sb[m][:, b * TQB:(b + 1) * TQB])
            nc.gpsimd.collective_compute(
                "ReduceScatter", AL.add, replica_groups=groups,
                ins=[rs_in[chn].opt()], outs=[rs_out[chn].opt()])

        # -- local token-quarter residual: xq += d1q (kept for delta AG) --
        dfq = [dfp.tile([128, TQ], BF, name=f"dfq{k}", tag=f"dfq{k}")
               for k in range(KT)]
        for c in range(NCH):
            qs = slice(c * TQB, (c + 1) * TQB)
            for k in range(KT):
                nc.sync.dma_start(out=dfq[k][:, qs],
                                  in_=rs_out[c][k * 128:(k + 1) * 128, :])
                nc.vector.tensor_tensor(out=xq[k][:, qs], in0=xq[k][:, qs],
                                        in1=dfq[k][:, qs], op=AL.add)

        # -- LN2 on local quarter + full-width MLP on 256 tokens --
        h2q = layernorm(xq, g2d[l], be2d[l], f"ln2_{l}", ncols=TQ, nch=NCH,
                        hpool=hq, htag="hq")
        uq = uqp.tile([128, KTF, TQ], BF, name="uq", tag="uq")
        for m in range(KTF):
            w1m = w1s.tile([128, KT, 128], BF, name="w1m", tag="w1m")
            nc.sync.dma_start(out=w1m, in_=w1[l][:, m])
            pu = psmm.tile([128, TQ], F32, name="psu", tag="mm")
            for k in range(KT):
                nc.tensor.matmul(pu, w1m[:, k, :], h2q[k],
                                 start=(k == 0), stop=(k == KT - 1))
            nc.scalar.activation(uq[:, m, :], pu, AF.Gelu,
                                 bias=b1col[:, m:m + 1])

        # -- W2 full-width on local tokens; delta = d1q + z + b2 --
        for m in range(KT):
            w2m = w1s.tile([128, KTF, 128], BF, name="w2m", tag="w2m")
            nc.sync.dma_start(out=w2m, in_=w2[l][:, m])
            psz = psmm.tile([128, TQ], F32, name="psz", tag="mm")
            for kk in range(KTF):
                nc.tensor.matmul(psz, w2m[:, kk, :], uq[:, kk, :],
                                 start=(kk == 0), stop=(kk == KTF - 1))
            nc.vector.scalar_tensor_tensor(
                out=xq[m], in0=psz, scalar=b2col[:, m:m + 1],
                in1=xq[m], op0=AL.add, op1=AL.add)
            nc.vector.scalar_tensor_tensor(
                out=dfq[m], in0=psz, scalar=b2col[:, m:m + 1],
                in1=dfq[m], op0=AL.add, op1=AL.add)

        # -- AllGather layer deltas per token half; update full x --
        ag_in = [dram.tile([D, TQB], BF, name=f"ag_in{l}_{c}")
                 for c in range(NCH)]
        ag_out = [dram.tile([TP, D, TQB], BF, name=f"ag_out{l}_{c}")
                  for c in range(NCH)]
        for c in range(NCH):
            qs = slice(c * TQB, (c + 1) * TQB)
            for k in range(KT):
                nc.sync.dma_start(out=ag_in[c][k * 128:(k + 1) * 128, :],
                                  in_=dfq[k][:, qs])
            nc.gpsimd.collective_compute(
                "AllGather", AL.bypass, replica_groups=groups,
                ins=[ag_in[c].opt()], outs=[ag_out[c].opt()])
        for c in range(NCH):
            for rr in range(TP):
                xf = agf.tile([128, KT, TQB], BF, name="xf", tag="agf")
                nc.sync.dma_start(
                    out=xf,
                    in_=ag_out[c][rr].rearrange("(k p) t -> p k t", p=128))
                tb = c * TCH + rr * TQB
                for k in range(KT):
                    nc.vector.tensor_tensor(
                        out=x[k][:, tb:tb + TQB], in0=x[k][:, tb:tb + TQB],
                        in1=xf[:, k, :], op=AL.add)

    # ---------------- final LN + logits ----------------
    hf = layernorm(x, gfd[0], befd[0], "lnf")
    for n in range(NV):
        hb = hwp.tile([128, KT, VCH], BF, name="hwb", tag="hwb")
        nc.sync.dma_start(out=hb, in_=hwd[n])
        for t in range(TT):
            ps = psmm.tile([128, TCH], F32, name="pslg", tag="mm")
            for k in range(KT):
                nc.tensor.matmul(ps[:, 0:VCH],
                                 hf[k][:, t * 128:(t + 1) * 128],
                                 hb[:, k, :],
                                 start=(k == 0), stop=(k == KT - 1))
            lg = lgp.tile([128, VCH], F32, name="lg", tag="lg")
            nc.vector.tensor_copy(lg, ps[:, 0:VCH])
            nc.sync.dma_start(
                out=logits[t * 128:(t + 1) * 128, n * VCH:(n + 1) * VCH],
                in_=lg)

    ctx.close()


# ---------------- host side ----------------

_PROG_CACHE = {}


def _get_program():
    if "nc" not in _PROG_CACHE:
        _PROG_CACHE["nc"] = build_program()
    return _PROG_CACHE["nc"]


def make_in_maps(input_ids, emb, Wq, Wk, Wv, Wo, W1, b1, W2, b2,
                 ln1_g, ln1_b, ln2_g, ln2_b, lnf_g, lnf_b, head_w):
    TP = CFG["TP"]
    D, V = CFG["D"], CFG["V"]
    DSH, DFS, VSH = D // TP, 4 * D // TP, V // TP
    bf = ml_dtypes.bfloat16
    in_maps = []
    S = CFG["S"]
    L = CFG["L"]
    TQB = S // TP // 2
    KT, KTF = D // 128, 4 * D // 128
    w1f = np.ascontiguousarray(
        np.asarray(W1).reshape(L, KT, 128, KTF, 128)
        .transpose(0, 2, 3, 1, 4)).astype(bf)
    w2f = np.ascontiguousarray(
        np.asarray(W2).reshape(L, KTF, 128, KT, 128)
        .transpose(0, 2, 3, 1, 4)).astype(bf)
    b1f = np.ascontiguousarray(b1).astype(np.float32)
    def _wtile(wfull):
        # [L, D, DSH] -> [L, 128, KT, DSH]
        a = np.asarray(wfull)
        return np.ascontiguousarray(
            a.reshape(L, KT, 128, a.shape[-1]).transpose(0, 2, 1, 3))
    for c in range(N_CORES):
        g, r = c // TP, c % TP
        x0 = np.asarray(emb)[np.asarray(input_ids)[g]]          # [S, D] f32
        x0T = np.ascontiguousarray(x0.T).astype(np.float32)
        xq0 = np.concatenate(
            [x0T[:, r * TQB:(r + 1) * TQB],
             x0T[:, S // 2 + r * TQB:S // 2 + (r + 1) * TQB]], axis=1)
        sel2 = np.zeros((65, 128), np.float32)
        sel2[0, 0:64] = 1.0
        sel2[64, 64:128] = 1.0
        in_maps.append({
            "sel2d": sel2,
            "xT0": x0T,
            "xq0": np.ascontiguousarray(xq0).astype(np.float32),
            "wq": _wtile(Wq[:, :, r * DSH:(r + 1) * DSH]).astype(bf),
            "wk": _wtile(Wk[:, :, r * DSH:(r + 1) * DSH]).astype(bf),
            "wv": _wtile(Wv[:, :, r * DSH:(r + 1) * DSH]).astype(bf),
            "wo": np.ascontiguousarray(
                np.asarray(Wo)[:, r * DSH:(r + 1) * DSH, :]
                .reshape(L, DSH // 128, 128, D)
                .transpose(0, 2, 1, 3)).astype(bf),
            "w1": w1f,
            "w2": w2f,
            "b1": b1f,
            "b2": np.asarray(b2, dtype=np.float32),
            "g1": np.asarray(ln1_g, dtype=np.float32),
            "be1": np.asarray(ln1_b, dtype=np.float32),
            "g2": np.asarray(ln2_g, dtype=np.float32),
            "be2": np.asarray(ln2_b, dtype=np.float32),
            "gf": np.asarray(lnf_g, dtype=np.float32).reshape(1, -1),
            "bef": np.asarray(lnf_b, dtype=np.float32).reshape(1, -1),
            "hw": np.ascontiguousarray(
                np.asarray(head_w)[:, r * VSH:(r + 1) * VSH]
                .reshape(KT, 128, 16, 500).transpose(2, 1, 0, 3)).astype(bf),
        })
    return in_maps


def kernel(**inputs):
    B, S, V = CFG["B"], CFG["S"], CFG["V"]
    TP = CFG["TP"]
    VSH = V // TP
    nc = _get_program()
    in_maps = make_in_maps(**inputs)
    res = run_bass_kernel_spmd(nc, in_maps, list(range(N_CORES)), trace=False)
    out = np.empty((B, S, V), dtype=np.float32)
    for c in range(N_CORES):
        g, r = c // TP, c % TP
        out[g, :, r * VSH:(r + 1) * VSH] = res.results[c]["logits"]
    return out


def run_traced(**inputs):
    """Like kernel() but with NTFF tracing; returns (out, exec_time_ns)."""
    nc = _get_program()
    in_maps = make_in_maps(**inputs)
    res = run_bass_kernel_spmd(nc, in_maps, list(range(N_CORES)), trace=True)
    B, S, V = CFG["B"], CFG["S"], CFG["V"]
    TP = CFG["TP"]
    VSH = V // TP
    out = np.empty((B, S, V), dtype=np.float32)
    for c in range(N_CORES):
        g, r = c // TP, c % TP
        out[g, :, r * VSH:(r + 1) * VSH] = res.results[c]["logits"]
    return out, res.exec_time_ns



# revision 29
# speedup vs baseline: 1.0440x; 1.0252x over previous
"""Bass/Tile kernel for a 4-layer dense transformer (prefill) on 8 TRN2 cores.

Parallelization: 2-way data parallel (batch) x 4-way tensor parallel.
Groups: cores [0,1,2,3] handle batch 0, [4,5,6,7] batch 1.
Within a group (rank r = core % 4):
  - attention: heads r*4..r*4+3  (feature cols r*256..(r+1)*256)
  - MLP: hidden cols r*1024..(r+1)*1024
  - vocab: cols r*8000..(r+1)*8000 of head_w
Activations are kept TRANSPOSED on device: [feature(partition), token(free)].
Residual stream x is fp32; matmul inputs are bf16 (fp32 PSUM accumulation).
Per layer: AllGather(attn-out bf16), AllGather(attn-delta fp32),
AllGather(mlp-hidden bf16), AllGather(mlp-delta fp32).
Final logits are computed in natural [token, vocab] layout and written out
per-core as [1024, 8000]; the host concatenates.
"""

import sys
import types

import numpy as np


def _install_ntff_shim():
    """Register the NTFF profiling hook that trn_boot skipped (the image's
    antenv package lacks the axon_hooks submodule)."""
    if "antenv.axon_hooks" in sys.modules:
        return
    try:
        import trn_agent_boot.trn_boot as tb
        hook = tb._ntff_profile_via_ctypes("/opt/axon/libaxon_pjrt.so")
    except Exception:
        hook = None
    mod = types.ModuleType("antenv.axon_hooks")
    _h = [hook]
    mod.get_axon_ntff_profile_hook = lambda: _h[0]
    mod.set_axon_ntff_profile_hook = lambda h: _h.__setitem__(0, h)
    sys.modules["antenv.axon_hooks"] = mod
    try:
        import antenv
        antenv.axon_hooks = mod
    except Exception:
        pass


_install_ntff_shim()

import ml_dtypes
import concourse.bass as bass
import concourse.mybir as mybir
import concourse.tile as tile
from concourse import bacc
from concourse.bass_utils import run_bass_kernel_spmd

BF = mybir.dt.bfloat16
F32 = mybir.dt.float32
AL = mybir.AluOpType
AF = mybir.ActivationFunctionType

# Model sizes (full problem, hardcoded per contract).
CFG = dict(
    B=2, S=1024, V=32000, D=1024, H=16, L=4, EPS=1e-5,
    TP=4,            # tensor-parallel width (group size)
    gelu_sim=False,  # CoreSim lacks Gelu; use sigmoid-based stand-in
)

N_CORES = 8
GROUPS = [[0, 1, 2, 3], [4, 5, 6, 7]]


def build_program(cfg=None):
    """Build the SPMD Bass program (identical on all 8 cores)."""
    c = dict(CFG)
    if cfg:
        c.update(cfg)
    B, S, V, D, H, L = c["B"], c["S"], c["V"], c["D"], c["H"], c["L"]
    EPS, TP = c["EPS"], c["TP"]
    T = S                    # tokens per group (one batch element)
    DK = D // H              # head dim (64)
    HL = H // TP             # heads per core (4)
    DSH = D // TP            # attention/delta feature shard (256)
    DF = 4 * D
    DFS = DF // TP           # mlp hidden shard (1024)
    VSH = V // TP            # vocab shard (8000)
    KT = D // 128            # feature k-tiles (8)
    KTF = DF // 128          # mlp k-tiles (32)
    NCH = max(1, T // 512)   # token chunks of <=512
    TCH = min(512, T)        # token chunk size
    MSH = DSH // 128         # m-tiles of a DSH-wide output (2)
    TKT = T // 128           # key-token tiles (8)
    VCH = 500                # vocab chunk
    NV = VSH // VCH          # vocab n-chunks (16)
    TT = T // 128            # token tiles (8)
    assert T % 128 == 0 and D % 128 == 0 and DSH % 128 == 0
    assert VSH % NV == 0 and VCH <= 512

    groups = [[g * TP + r for r in range(TP)] for g in range(N_CORES // TP)]

    nc = bacc.Bacc("TRN2", target_bir_lowering=False, debug=False,
                   num_devices=N_CORES)

    # ---- DRAM parameters (per-core shards fed via in_maps) ----
    xT0 = nc.dram_tensor("xT0", [D, T], F32, kind="ExternalInput")
    xq0 = nc.dram_tensor("xq0", [D, T // TP], F32, kind="ExternalInput")
    wq = nc.dram_tensor("wq", [L, 128, KT, DSH], BF, kind="ExternalInput")
    wk = nc.dram_tensor("wk", [L, 128, KT, DSH], BF, kind="ExternalInput")
    wv = nc.dram_tensor("wv", [L, 128, KT, DSH], BF, kind="ExternalInput")
    wo = nc.dram_tensor("wo", [L, 128, DSH // 128, D], BF, kind="ExternalInput")
    w1 = nc.dram_tensor("w1", [L, 128, KTF, KT, 128], BF, kind="ExternalInput")
    w2 = nc.dram_tensor("w2", [L, 128, KT, KTF, 128], BF, kind="ExternalInput")
    b1 = nc.dram_tensor("b1", [L, DF], F32, kind="ExternalInput")
    b2 = nc.dram_tensor("b2", [L, D], F32, kind="ExternalInput")
    g1 = nc.dram_tensor("g1", [L, D], F32, kind="ExternalInput")
    be1 = nc.dram_tensor("be1", [L, D], F32, kind="ExternalInput")
    g2 = nc.dram_tensor("g2", [L, D], F32, kind="ExternalInput")
    be2 = nc.dram_tensor("be2", [L, D], F32, kind="ExternalInput")
    gf = nc.dram_tensor("gf", [1, D], F32, kind="ExternalInput")
    bef = nc.dram_tensor("bef", [1, D], F32, kind="ExternalInput")
    hw = nc.dram_tensor("hw", [NV, 128, KT, VCH], BF, kind="ExternalInput")
    sel2d = nc.dram_tensor("sel2d", [65, 128], F32, kind="ExternalInput")
    logits = nc.dram_tensor("logits", [T, VSH], F32, kind="ExternalOutput")

    with tile.TileContext(nc) as tc:
        _build_tc(nc, tc, locals())
    nc.compile()
    return nc


def _build_tc(nc, tc, v):
    """Emit the tile program. `v` is the name->value dict from build_program."""
    (B, T, D, L, EPS, TP, DK, HL, DSH, DF, DFS, VSH, KT, KTF, NCH, TCH,
     MSH, TKT, NV, VCH, TT, groups) = (
        v["B"], v["T"], v["D"], v["L"], v["EPS"], v["TP"], v["DK"], v["HL"],
        v["DSH"], v["DF"], v["DFS"], v["VSH"], v["KT"], v["KTF"], v["NCH"],
        v["TCH"], v["MSH"], v["TKT"], v["NV"], v["VCH"], v["TT"], v["groups"])
    xT0, wq, wk, wv, wo, w1, w2 = (v["xT0"], v["wq"], v["wk"], v["wv"],
                                   v["wo"], v["w1"], v["w2"])
    b1d, b2d, g1d, be1d, g2d, be2d, gfd, befd = (
        v["b1"], v["b2"], v["g1"], v["be1"], v["g2"], v["be2"], v["gf"],
        v["bef"])
    hwd, logits = v["hw"], v["logits"]

    import contextlib
    ctx = contextlib.ExitStack()

    # ---------------- pools ----------------
    sing = ctx.enter_context(tc.tile_pool(name="sing", bufs=1))
    wts = ctx.enter_context(tc.tile_pool(name="wts", bufs=1))
    w1s = ctx.enter_context(tc.tile_pool(name="w1s", bufs=2))
    hwp = ctx.enter_context(tc.tile_pool(name="hwp", bufs=2))
    hp = ctx.enter_context(tc.tile_pool(name="hp", bufs=1))
    hq = ctx.enter_context(tc.tile_pool(name="hq", bufs=1))
    qkp = ctx.enter_context(tc.tile_pool(name="qkp", bufs=1))
    scr = ctx.enter_context(tc.tile_pool(name="scr", bufs=2))
    expp = ctx.enter_context(tc.tile_pool(name="expp", bufs=8))
    otp = ctx.enter_context(tc.tile_pool(name="otp", bufs=1))
    agf = ctx.enter_context(tc.tile_pool(name="agf", bufs=2))   # AG reads
    d8 = ctx.enter_context(tc.tile_pool(name="d8", bufs=1))     # attn delta
    dfp = ctx.enter_context(tc.tile_pool(name="dfp", bufs=1))   # RS-out quarter
    uqp = ctx.enter_context(tc.tile_pool(name="uqp", bufs=1))   # mlp hidden q
    lgp = ctx.enter_context(tc.tile_pool(name="lgp", bufs=2))
    tiny = ctx.enter_context(tc.tile_pool(name="tiny", bufs=2))
    rows1 = ctx.enter_context(tc.tile_pool(name="rows1", bufs=1))
    rows3 = ctx.enter_context(tc.tile_pool(name="rows3", bufs=2))
    rows2 = ctx.enter_context(tc.tile_pool(name="rows2", bufs=1))
    bcp = ctx.enter_context(tc.tile_pool(name="bcp", bufs=1))
    rbp = ctx.enter_context(tc.tile_pool(name="rbp", bufs=1))
    psmm = ctx.enter_context(tc.tile_pool(name="psmm", bufs=4, space="PSUM"))
    psaux = ctx.enter_context(tc.tile_pool(name="psaux", bufs=3, space="PSUM"))
    psst = ctx.enter_context(tc.tile_pool(name="psst", bufs=1, space="PSUM"))
    dram = ctx.enter_context(tc.tile_pool(name="dram", bufs=1, space="DRAM"))

    # ---------------- constants ----------------
    ones_col = sing.tile([128, 1], BF, name="ones_col")
    nc.vector.memset(ones_col, 1.0)
    ones_row = sing.tile([1, 128], BF, name="ones_row")
    nc.vector.memset(ones_row, 1.0)
    ones_row_f = sing.tile([1, 128], F32, name="ones_row_f")
    nc.vector.memset(ones_row_f, 1.0)
    eps_ap = sing.tile([1, 1], F32, name="eps_ap")
    nc.vector.memset(eps_ap, EPS)
    sel2 = sing.tile([65, 128], F32, name="sel2")
    nc.sync.dma_start(out=sel2, in_=v["sel2d"][:, :])
    # causal diagonal-band masks, one per key-block offset (built once)
    maskt = sing.tile([128, 4, TCH], BF, name="maskt")
    nc.vector.memset(maskt, 1.0)
    for o in range(4):
        nc.gpsimd.affine_select(
            out=maskt[:, o, :], in_=maskt[:, o, :], pattern=[[1, TCH]],
            compare_op=AL.is_ge, fill=0.0, base=-(o * 128),
            channel_multiplier=-1)

    # ---------------- residual stream ----------------
    TQ = T // TP                 # local MLP tokens per rank (256)
    TQB = TQ // NCH              # per token-half block (128)
    x = [sing.tile([128, T], F32, name=f"x{k}") for k in range(KT)]
    xq = [sing.tile([128, TQ], F32, name=f"xq{k}") for k in range(KT)]
    for k in range(KT):
        nc.sync.dma_start(out=x[k], in_=xT0[k * 128:(k + 1) * 128, :])
        nc.sync.dma_start(out=xq[k], in_=v["xq0"][k * 128:(k + 1) * 128, :])

    # ---------------- layernorm ----------------
    def layernorm(x_tiles, grow_dram, brow_dram, name, ncols=None, nch=None,
                  hpool=None, htag="h"):
        """LN over the feature (partition) axis of transposed activations.
        Chunk-outer so chunk 0 proceeds while chunk 1's inputs are still
        being gathered. Returns bf16 tiles h[kt] = LN(x)."""
        ncols = T if ncols is None else ncols
        nch = NCH if nch is None else nch
        hpool = hp if hpool is None else hpool
        tchl = ncols // nch
        # per-partition gamma/beta columns: [128, KT]
        gcol = tiny.tile([128, KT], F32, name=f"g_{name}")
        bcol = tiny.tile([128, KT], F32, name=f"b_{name}")
        nc.sync.dma_start(out=gcol, in_=grow_dram.rearrange("(k p) -> p k", p=128))
        nc.sync.dma_start(out=bcol, in_=brow_dram.rearrange("(k p) -> p k", p=128))

        h = [hpool.tile([128, ncols], BF, name=f"h_{name}_{k}",
                        tag=f"{htag}{k}")
             for k in range(KT)]
        for ch in range(nch):
            cs = slice(ch * tchl, (ch + 1) * tchl)
            # stats: PSUM tile holds sum at partition 0, sumsq at 32.
            ps_st = psst.tile([33, tchl], F32, name="ps_st", tag="ps_st")
            for k in range(KT):
                xbt = scr.tile([128, tchl], BF, name="xb", tag="xb")
                nc.vector.tensor_copy(xbt, x_tiles[k][:, cs])
                sqt = scr.tile([128, tchl], BF, name="sq", tag="sq")
                nc.scalar.square(sqt, xbt)
                nc.tensor.matmul(ps_st[0:1, :], ones_col, xbt,
                                 start=(k == 0), stop=(k == KT - 1))
                nc.tensor.matmul(ps_st[32:33, :], ones_col, sqt,
                                 start=(k == 0), stop=(k == KT - 1))
            # moments for this chunk
            st_sb = rows1.tile([1, 2 * tchl], F32, name=f"st_{name}", tag="st_sb")
            nc.vector.tensor_copy(st_sb[:, 0:tchl], ps_st[0:1, :])
            nc.vector.tensor_copy(st_sb[:, tchl:2 * tchl], ps_st[32:33, :])
            mom = rows1.tile([1, 2 * tchl], F32, name=f"mom_{name}", tag="mom")
            nc.scalar.mul(mom, st_sb, 1.0 / D)      # [mean | E[x^2]]
            mean = mom[:, 0:tchl]
            msq = mom[:, tchl:2 * tchl]
            m2 = rows3.tile([1, tchl], F32, name=f"m2_{name}", tag="row1k")
            nc.vector.tensor_mul(m2, mean, mean)
            var = rows3.tile([1, tchl], F32, name=f"var_{name}", tag="row1k")
            nc.vector.tensor_tensor(out=var, in0=msq, in1=m2, op=AL.subtract)
            sd = rows3.tile([1, tchl], F32, name=f"sd_{name}", tag="row1k")
            nc.scalar.activation(sd, var, AF.Sqrt, bias=eps_ap)
            rstd = rows3.tile([1, tchl], F32, name=f"rstd_{name}", tag="row1k")
            nc.vector.reciprocal(rstd, sd)
            nmr = rows3.tile([1, tchl], F32, name=f"nmr_{name}", tag="row1k")
            nc.vector.tensor_mul(nmr, mean, rstd)
            nc.scalar.mul(nmr, nmr, -1.0)           # -mean*rstd
            # broadcast to [128, tchl] via K=1 outer-product matmuls (fp32)
            rstdB = bcp.tile([128, tchl], F32, name="rstdB", tag="rstdB")
            nmB = bcp.tile([128, tchl], F32, name="nmB", tag="nmB")
            pb = psaux.tile([128, tchl], F32, name="pb", tag="aux")
            nc.tensor.matmul(pb, ones_row_f, rstd, start=True, stop=True)
            nc.scalar.copy(rstdB, pb)
            pb2 = psaux.tile([128, tchl], F32, name="pb2", tag="aux")
            nc.tensor.matmul(pb2, ones_row_f, nmr, start=True, stop=True)
            nc.scalar.copy(nmB, pb2)
            # apply: h = (x*rstdB + nmB)*g + b, output bf16
            for k in range(KT):
                t1 = scr.tile([128, tchl], BF, name="lnt", tag="lnt")
                nc.vector.tensor_mul(t1, x_tiles[k][:, cs], rstdB)
                t2 = scr.tile([128, tchl], BF, name="lnt2", tag="lnt2")
                nc.vector.tensor_tensor(out=t2, in0=t1, in1=nmB, op=AL.add)
                nc.vector.tensor_scalar(
                    out=h[k][:, cs], in0=t2, scalar1=gcol[:, k:k + 1],
                    scalar2=bcol[:, k:k + 1], op0=AL.mult, op1=AL.add)
        return h

    # ---------------- transformer layers ----------------
    for l in range(L):
        # -- weights for this layer --
        wqt = wts.tile([128, KT, DSH], BF, name="wqt", tag="wqt")
        wkt = wts.tile([128, KT, DSH], BF, name="wkt", tag="wkt")
        wvt = wts.tile([128, KT, DSH], BF, name="wvt", tag="wvt")
        for dst, srcw in ((wqt, wq), (wkt, wk), (wvt, wv)):
            nc.scalar.dma_start(out=dst, in_=srcw[l])
        # row-sharded Wo: [DSH local head feats, D] -> [128, 2, D]
        wot = wts.tile([128, MSH, D], BF, name="wot", tag="wot")
        nc.scalar.dma_start(out=wot, in_=wo[l])
        b1col = tiny.tile([128, KTF], F32, name="b1col", tag="b1col")
        nc.sync.dma_start(out=b1col, in_=b1d[l].rearrange("(k p) -> p k", p=128))
        b2col = tiny.tile([128, KT], F32, name="b2col", tag="b2col")
        nc.sync.dma_start(out=b2col, in_=b2d[l].rearrange("(k p) -> p k", p=128))

        # -- LN1 --
        h1 = layernorm(x, g1d[l], be1d[l], f"ln1_{l}")

        # -- QKV projections (chunk-outer so attention c0 starts early) --
        # qT/kT: [DSH, T] transposed; v: natural [T, DSH] + ones column
        qT = [qkp.tile([128, T], BF, name=f"qT{m}", tag=f"qT{m}")
              for m in range(MSH)]
        kTt = [qkp.tile([128, T], BF, name=f"kT{m}", tag=f"kT{m}")
               for m in range(MSH)]
        vt = qkp.tile([128, TKT, HL, DK + 1], BF, name="vt", tag="vt")
        nc.vector.memset(vt[:, :, :, DK:DK + 1], 1.0)
        for chn in range(NCH):
            cs = slice(chn * TCH, (chn + 1) * TCH)
            for wt, dst in ((wkt, kTt), (wqt, qT)):
                pq = {}
                for m in range(MSH):
                    pq[m] = psmm.tile([128, TCH], F32, name="ps", tag="mm")
                for k in range(KT):
                    for m in range(MSH):
                        nc.tensor.matmul(pq[m],
                                         wt[:, k, m * 128:(m + 1) * 128],
                                         h1[k][:, cs],
                                         start=(k == 0), stop=(k == KT - 1))
                for m in range(MSH):
                    nc.vector.tensor_copy(dst[m][:, cs], pq[m])
            for t in range(chn * (TCH // 128), (chn + 1) * (TCH // 128)):
                ps = psmm.tile([128, TCH], F32, name="psv", tag="mm")
                for k in range(KT):
                    nc.tensor.matmul(ps[:, 0:DSH],
                                     h1[k][:, t * 128:(t + 1) * 128],
                                     wvt[:, k, :],
                                     start=(k == 0), stop=(k == KT - 1))
                nc.vector.tensor_copy(
                    vt[:, t, :, 0:DK],
                    ps[:, 0:DSH].rearrange("p (h d) -> p h d", h=HL))

        # -- attention, chunk-outer; Wo row-shard -> token ReduceScatter --
        rs_in = [dram.tile([TP, D, TQB], BF, name=f"rs_in{l}_{c}")
                 for c in range(NCH)]
        rs_out = [dram.tile([D, TQB], BF, name=f"rs_out{l}_{c}")
                  for c in range(NCH)]
        for chn in range(NCH):
            cs = slice(chn * TCH, (chn + 1) * TCH)
            jmax = (chn + 1) * (TCH // 128)
            oT = [otp.tile([128, TCH], BF, name=f"oT{m}", tag=f"oT{m}")
                  for m in range(MSH)]
            for hpi in range(HL // 2):
                # heads (2hp, 2hp+1) sit at partitions 0-63 / 64-127 of
                # m-tile hp: their K=64 score matmuls land in different PE
                # row-groups and run concurrently.
                mt = hpi
                ps_os = [psaux.tile([DK + 1, TCH], F32, name="ps_o",
                                    tag="aux") for _ in range(2)]
                for j in range(jmax):
                    ets = []
                    for sub in range(2):
                        po = sub * DK
                        pss = psmm.tile([128, TCH], F32, name="pss",
                                        tag="mm")
                        nc.tensor.matmul(
                            pss, kTt[mt][po:po + DK, j * 128:(j + 1) * 128],
                            qT[mt][po:po + DK, cs], start=True, stop=True)
                        et = expp.tile([128, TCH], BF, name="exp", tag="exp")
                        nc.scalar.activation(et, pss, AF.Exp, scale=0.125)
                        if j * 128 >= chn * TCH:
                            # zero where tk_global > tq_global
                            nc.vector.tensor_mul(
                                et, et, maskt[:, j - chn * 4, :])
                        ets.append(et)
                    for sub in range(2):
                        nc.tensor.matmul(ps_os[sub], vt[:, j, 2 * hpi + sub, :],
                                         ets[sub],
                                         start=(j == 0), stop=(j == jmax - 1))
                den2 = rows2.tile([65, TCH], F32, name="den2", tag="den")
                nc.vector.memset(den2, 1.0)
                nc.vector.tensor_copy(den2[0:1, :], ps_os[0][DK:DK + 1, :])
                nc.vector.tensor_copy(den2[DK:DK + 1, :],
                                      ps_os[1][DK:DK + 1, :])
                rec2 = rows2.tile([65, TCH], F32, name="rec2", tag="rec")
                rsc2 = rows2.tile([65, TCH], F32, name="rsc2", tag="rsc")
                nc.vector.reciprocal_approx_accurate(rec2, den2, rsc2)
                ps_r = psmm.tile([128, TCH], F32, name="ps_r", tag="mm")
                nc.tensor.matmul(ps_r, sel2, rec2, start=True, stop=True)
                rb = rbp.tile([128, TCH], BF, name="rb", tag="rb")
                nc.scalar.copy(rb, ps_r)
                for sub in range(2):
                    po = sub * DK
                    nc.vector.tensor_tensor(
                        out=oT[mt][po:po + DK, :], in0=ps_os[sub][0:DK, :],
                        in1=rb[po:po + DK, :], op=AL.mult)
            # Wo row-shard: d1_part[m] = Wo[local rows].T @ o_local, full D
            d1_sb = [d8.tile([128, TCH], BF, name=f"d1s{m}", tag=f"d1s{m}")
                     for m in range(KT)]
            for m in range(KT):
                psd = psmm.tile([128, TCH], F32, name="psd1", tag="mm")
                for kk in range(MSH):
                    nc.tensor.matmul(psd,
                                     wot[:, kk, m * 128:(m + 1) * 128],
                                     oT[kk],
                                     start=(kk == 0), stop=(kk == MSH - 1))
                nc.vector.tensor_copy(d1_sb[m], psd)
                for b in range(TCH // TQB):
                    nc.sync.dma_start(
                        out=rs_in[chn][b, m * 128:(m + 1) * 128, :],
                        in_=d1_sb[m][:, b * TQB:(b + 1) * TQB])
            nc.gpsimd.collective_compute(
                "ReduceScatter", AL.add, replica_groups=groups,
                ins=[rs_in[chn].opt()], outs=[rs_out[chn].opt()])

        # -- local token-quarter residual: xq += d1q (kept for delta AG) --
        dfq = [dfp.tile([128, TQ], BF, name=f"dfq{k}", tag=f"dfq{k}")
               for k in range(KT)]
        for c in range(NCH):
            qs = slice(c * TQB, (c + 1) * TQB)
            for k in range(KT):
                nc.sync.dma_start(out=dfq[k][:, qs],
                                  in_=rs_out[c][k * 128:(k + 1) * 128, :])
                nc.vector.tensor_tensor(out=xq[k][:, qs], in0=xq[k][:, qs],
                                        in1=dfq[k][:, qs], op=AL.add)

        # -- LN2 on local quarter + full-width MLP on 256 tokens --
        h2q = layernorm(xq, g2d[l], be2d[l], f"ln2_{l}", ncols=TQ, nch=NCH,
                        hpool=hq, htag="hq")
        uq = uqp.tile([128, KTF, TQ], BF, name="uq", tag="uq")
        for m in range(KTF):
            w1m = w1s.tile([128, KT, 128], BF, name="w1m", tag="w1m")
            nc.scalar.dma_start(out=w1m, in_=w1[l][:, m])
            pu = psmm.tile([128, TQ], F32, name="psu", tag="mm")
            for k in range(KT):
                nc.tensor.matmul(pu, w1m[:, k, :], h2q[k],
                                 start=(k == 0), stop=(k == KT - 1))
            nc.scalar.activation(uq[:, m, :], pu, AF.Gelu,
                                 bias=b1col[:, m:m + 1])

        # -- W2 full-width on local tokens; delta = d1q + z + b2 --
        for m in range(KT):
            w2m = w1s.tile([128, KTF, 128], BF, name="w2m", tag="w2m")
            nc.scalar.dma_start(out=w2m, in_=w2[l][:, m])
            psz = psmm.tile([128, TQ], F32, name="psz", tag="mm")
            for kk in range(KTF):
                nc.tensor.matmul(psz, w2m[:, kk, :], uq[:, kk, :],
                                 start=(kk == 0), stop=(kk == KTF - 1))
            nc.vector.scalar_tensor_tensor(
                out=xq[m], in0=psz, scalar=b2col[:, m:m + 1],
                in1=xq[m], op0=AL.add, op1=AL.add)
            nc.vector.scalar_tensor_tensor(
                out=dfq[m], in0=psz, scalar=b2col[:, m:m + 1],
                in1=dfq[m], op0=AL.add, op1=AL.add)

        # -- AllGather layer deltas per token half; update full x --
        ag_in = [dram.tile([D, TQB], BF, name=f"ag_in{l}_{c}")
                 for c in range(NCH)]
        ag_out = [dram.tile([TP, D, TQB], BF, name=f"ag_out{l}_{c}")
                  for c in range(NCH)]
        for c in range(NCH):
            qs = slice(c * TQB, (c + 1) * TQB)
            for k in range(KT):
                nc.sync.dma_start(out=ag_in[c][k * 128:(k + 1) * 128, :],
                                  in_=dfq[k][:, qs])
            nc.gpsimd.collective_compute(
                "AllGather", AL.bypass, replica_groups=groups,
                ins=[ag_in[c].opt()], outs=[ag_out[c].opt()])
        for c in range(NCH):
            for rr in range(TP):
                xf = agf.tile([128, KT, TQB], BF, name="xf", tag="agf")
                nc.sync.dma_start(
                    out=xf,
                    in_=ag_out[c][rr].rearrange("(k p) t -> p k t", p=128))
                tb = c * TCH + rr * TQB
                for k in range(KT):
                    nc.vector.tensor_tensor(
                        out=x[k][:, tb:tb + TQB], in0=x[k][:, tb:tb + TQB],
                        in1=xf[:, k, :], op=AL.add)

    # ---------------- final LN + logits ----------------
    hf = layernorm(x, gfd[0], befd[0], "lnf")
    for n in range(NV):
        hb = hwp.tile([128, KT, VCH], BF, name="hwb", tag="hwb")
        nc.scalar.dma_start(out=hb, in_=hwd[n])
        for t in range(TT):
            ps = psmm.tile([128, TCH], F32, name="pslg", tag="mm")
            for k in range(KT):
                nc.tensor.matmul(ps[:, 0:VCH],
                                 hf[k][:, t * 128:(t + 1) * 128],
                                 hb[:, k, :],
                                 start=(k == 0), stop=(k == KT - 1))
            lg = lgp.tile([128, VCH], F32, name="lg", tag="lg")
            nc.vector.tensor_copy(lg, ps[:, 0:VCH])
            nc.sync.dma_start(
                out=logits[t * 128:(t + 1) * 128, n * VCH:(n + 1) * VCH],
                in_=lg)

    ctx.close()


# ---------------- host side ----------------

_PROG_CACHE = {}


def _get_program():
    if "nc" not in _PROG_CACHE:
        _PROG_CACHE["nc"] = build_program()
    return _PROG_CACHE["nc"]


def make_in_maps(input_ids, emb, Wq, Wk, Wv, Wo, W1, b1, W2, b2,
                 ln1_g, ln1_b, ln2_g, ln2_b, lnf_g, lnf_b, head_w):
    TP = CFG["TP"]
    D, V = CFG["D"], CFG["V"]
    DSH, DFS, VSH = D // TP, 4 * D // TP, V // TP
    bf = ml_dtypes.bfloat16
    in_maps = []
    S = CFG["S"]
    L = CFG["L"]
    TQB = S // TP // 2
    KT, KTF = D // 128, 4 * D // 128
    w1f = np.ascontiguousarray(
        np.asarray(W1).reshape(L, KT, 128, KTF, 128)
        .transpose(0, 2, 3, 1, 4)).astype(bf)
    w2f = np.ascontiguousarray(
        np.asarray(W2).reshape(L, KTF, 128, KT, 128)
        .transpose(0, 2, 3, 1, 4)).astype(bf)
    b1f = np.ascontiguousarray(b1).astype(np.float32)
    def _wtile(wfull):
        # [L, D, DSH] -> [L, 128, KT, DSH]
        a = np.asarray(wfull)
        return np.ascontiguousarray(
            a.reshape(L, KT, 128, a.shape[-1]).transpose(0, 2, 1, 3))
    for c in range(N_CORES):
        g, r = c // TP, c % TP
        x0 = np.asarray(emb)[np.asarray(input_ids)[g]]          # [S, D] f32
        x0T = np.ascontiguousarray(x0.T).astype(np.float32)
        xq0 = np.concatenate(
            [x0T[:, r * TQB:(r + 1) * TQB],
             x0T[:, S // 2 + r * TQB:S // 2 + (r + 1) * TQB]], axis=1)
        sel2 = np.zeros((65, 128), np.float32)
        sel2[0, 0:64] = 1.0
        sel2[64, 64:128] = 1.0
        in_maps.append({
            "sel2d": sel2,
            "xT0": x0T,
            "xq0": np.ascontiguousarray(xq0).astype(np.float32),
            "wq": _wtile(Wq[:, :, r * DSH:(r + 1) * DSH]).astype(bf),
            "wk": _wtile(Wk[:, :, r * DSH:(r + 1) * DSH]).astype(bf),
            "wv": _wtile(Wv[:, :, r * DSH:(r + 1) * DSH]).astype(bf),
            "wo": np.ascontiguousarray(
                np.asarray(Wo)[:, r * DSH:(r + 1) * DSH, :]
                .reshape(L, DSH // 128, 128, D)
                .transpose(0, 2, 1, 3)).astype(bf),
            "w1": w1f,
            "w2": w2f,
            "b1": b1f,
            "b2": np.asarray(b2, dtype=np.float32),
            "g1": np.asarray(ln1_g, dtype=np.float32),
            "be1": np.asarray(ln1_b, dtype=np.float32),
            "g2": np.asarray(ln2_g, dtype=np.float32),
            "be2": np.asarray(ln2_b, dtype=np.float32),
            "gf": np.asarray(lnf_g, dtype=np.float32).reshape(1, -1),
            "bef": np.asarray(lnf_b, dtype=np.float32).reshape(1, -1),
            "hw": np.ascontiguousarray(
                np.asarray(head_w)[:, r * VSH:(r + 1) * VSH]
                .reshape(KT, 128, 16, 500).transpose(2, 1, 0, 3)).astype(bf),
        })
    return in_maps


def kernel(**inputs):
    B, S, V = CFG["B"], CFG["S"], CFG["V"]
    TP = CFG["TP"]
    VSH = V // TP
    nc = _get_program()
    in_maps = make_in_maps(**inputs)
    res = run_bass_kernel_spmd(nc, in_maps, list(range(N_CORES)), trace=False)
    out = np.empty((B, S, V), dtype=np.float32)
    for c in range(N_CORES):
        g, r = c // TP, c % TP
        out[g, :, r * VSH:(r + 1) * VSH] = res.results[c]["logits"]
    return out


def run_traced(**inputs):
    """Like kernel() but with NTFF tracing; returns (out, exec_time_ns)."""
    nc = _get_program()
    in_maps = make_in_maps(**inputs)
    res = run_bass_kernel_spmd(nc, in_maps, list(range(N_CORES)), trace=True)
    B, S, V = CFG["B"], CFG["S"], CFG["V"]
    TP = CFG["TP"]
    VSH = V // TP
    out = np.empty((B, S, V), dtype=np.float32)
    for c in range(N_CORES):
        g, r = c // TP, c % TP
        out[g, :, r * VSH:(r + 1) * VSH] = res.results[c]["logits"]
    return out, res.exec_time_ns



# revision 30
# speedup vs baseline: 1.0553x; 1.0108x over previous
"""Bass/Tile kernel for a 4-layer dense transformer (prefill) on 8 TRN2 cores.

Parallelization: 2-way data parallel (batch) x 4-way tensor parallel.
Groups: cores [0,1,2,3] handle batch 0, [4,5,6,7] batch 1.
Within a group (rank r = core % 4):
  - attention: heads r*4..r*4+3  (feature cols r*256..(r+1)*256)
  - MLP: hidden cols r*1024..(r+1)*1024
  - vocab: cols r*8000..(r+1)*8000 of head_w
Activations are kept TRANSPOSED on device: [feature(partition), token(free)].
Residual stream x is fp32; matmul inputs are bf16 (fp32 PSUM accumulation).
Per layer: AllGather(attn-out bf16), AllGather(attn-delta fp32),
AllGather(mlp-hidden bf16), AllGather(mlp-delta fp32).
Final logits are computed in natural [token, vocab] layout and written out
per-core as [1024, 8000]; the host concatenates.
"""

import sys
import types

import numpy as np


def _install_ntff_shim():
    """Register the NTFF profiling hook that trn_boot skipped (the image's
    antenv package lacks the axon_hooks submodule)."""
    if "antenv.axon_hooks" in sys.modules:
        return
    try:
        import trn_agent_boot.trn_boot as tb
        hook = tb._ntff_profile_via_ctypes("/opt/axon/libaxon_pjrt.so")
    except Exception:
        hook = None
    mod = types.ModuleType("antenv.axon_hooks")
    _h = [hook]
    mod.get_axon_ntff_profile_hook = lambda: _h[0]
    mod.set_axon_ntff_profile_hook = lambda h: _h.__setitem__(0, h)
    sys.modules["antenv.axon_hooks"] = mod
    try:
        import antenv
        antenv.axon_hooks = mod
    except Exception:
        pass


_install_ntff_shim()

import ml_dtypes
import concourse.bass as bass
import concourse.mybir as mybir
import concourse.tile as tile
from concourse import bacc
from concourse.bass_utils import run_bass_kernel_spmd

BF = mybir.dt.bfloat16
F32 = mybir.dt.float32
AL = mybir.AluOpType
AF = mybir.ActivationFunctionType

# Model sizes (full problem, hardcoded per contract).
CFG = dict(
    B=2, S=1024, V=32000, D=1024, H=16, L=4, EPS=1e-5,
    TP=4,            # tensor-parallel width (group size)
    gelu_sim=False,  # CoreSim lacks Gelu; use sigmoid-based stand-in
)

N_CORES = 8
GROUPS = [[0, 1, 2, 3], [4, 5, 6, 7]]


def build_program(cfg=None):
    """Build the SPMD Bass program (identical on all 8 cores)."""
    c = dict(CFG)
    if cfg:
        c.update(cfg)
    B, S, V, D, H, L = c["B"], c["S"], c["V"], c["D"], c["H"], c["L"]
    EPS, TP = c["EPS"], c["TP"]
    T = S                    # tokens per group (one batch element)
    DK = D // H              # head dim (64)
    HL = H // TP             # heads per core (4)
    DSH = D // TP            # attention/delta feature shard (256)
    DF = 4 * D
    DFS = DF // TP           # mlp hidden shard (1024)
    VSH = V // TP            # vocab shard (8000)
    KT = D // 128            # feature k-tiles (8)
    KTF = DF // 128          # mlp k-tiles (32)
    NCH = max(1, T // 512)   # token chunks of <=512
    TCH = min(512, T)        # token chunk size
    MSH = DSH // 128         # m-tiles of a DSH-wide output (2)
    TKT = T // 128           # key-token tiles (8)
    VCH = 500                # vocab chunk
    NV = VSH // VCH          # vocab n-chunks (16)
    TT = T // 128            # token tiles (8)
    assert T % 128 == 0 and D % 128 == 0 and DSH % 128 == 0
    assert VSH % NV == 0 and VCH <= 512

    groups = [[g * TP + r for r in range(TP)] for g in range(N_CORES // TP)]

    nc = bacc.Bacc("TRN2", target_bir_lowering=False, debug=False,
                   num_devices=N_CORES)

    # ---- DRAM parameters (per-core shards fed via in_maps) ----
    xT0 = nc.dram_tensor("xT0", [D, T], F32, kind="ExternalInput")
    xq0 = nc.dram_tensor("xq0", [D, T // TP], F32, kind="ExternalInput")
    wq = nc.dram_tensor("wq", [L, 128, KT, DSH], BF, kind="ExternalInput")
    wk = nc.dram_tensor("wk", [L, 128, KT, DSH], BF, kind="ExternalInput")
    wv = nc.dram_tensor("wv", [L, 128, KT, DSH], BF, kind="ExternalInput")
    wo = nc.dram_tensor("wo", [L, 128, DSH // 128, D], BF, kind="ExternalInput")
    w1 = nc.dram_tensor("w1", [L, 128, KTF, KT, 128], BF, kind="ExternalInput")
    w2 = nc.dram_tensor("w2", [L, 128, KT, KTF, 128], BF, kind="ExternalInput")
    b1 = nc.dram_tensor("b1", [L, DF], F32, kind="ExternalInput")
    b2 = nc.dram_tensor("b2", [L, D], F32, kind="ExternalInput")
    g1 = nc.dram_tensor("g1", [L, D], F32, kind="ExternalInput")
    be1 = nc.dram_tensor("be1", [L, D], F32, kind="ExternalInput")
    g2 = nc.dram_tensor("g2", [L, D], F32, kind="ExternalInput")
    be2 = nc.dram_tensor("be2", [L, D], F32, kind="ExternalInput")
    gf = nc.dram_tensor("gf", [1, D], F32, kind="ExternalInput")
    bef = nc.dram_tensor("bef", [1, D], F32, kind="ExternalInput")
    hw = nc.dram_tensor("hw", [NV, 128, KT, VCH], BF, kind="ExternalInput")
    sel2d = nc.dram_tensor("sel2d", [65, 128], F32, kind="ExternalInput")
    logits = nc.dram_tensor("logits", [T, VSH], F32, kind="ExternalOutput")

    with tile.TileContext(nc) as tc:
        _build_tc(nc, tc, locals())
    nc.compile()
    return nc


def _build_tc(nc, tc, v):
    """Emit the tile program. `v` is the name->value dict from build_program."""
    (B, T, D, L, EPS, TP, DK, HL, DSH, DF, DFS, VSH, KT, KTF, NCH, TCH,
     MSH, TKT, NV, VCH, TT, groups) = (
        v["B"], v["T"], v["D"], v["L"], v["EPS"], v["TP"], v["DK"], v["HL"],
        v["DSH"], v["DF"], v["DFS"], v["VSH"], v["KT"], v["KTF"], v["NCH"],
        v["TCH"], v["MSH"], v["TKT"], v["NV"], v["VCH"], v["TT"], v["groups"])
    xT0, wq, wk, wv, wo, w1, w2 = (v["xT0"], v["wq"], v["wk"], v["wv"],
                                   v["wo"], v["w1"], v["w2"])
    b1d, b2d, g1d, be1d, g2d, be2d, gfd, befd = (
        v["b1"], v["b2"], v["g1"], v["be1"], v["g2"], v["be2"], v["gf"],
        v["bef"])
    hwd, logits = v["hw"], v["logits"]

    import contextlib
    ctx = contextlib.ExitStack()

    # ---------------- pools ----------------
    sing = ctx.enter_context(tc.tile_pool(name="sing", bufs=1))
    wts = ctx.enter_context(tc.tile_pool(name="wts", bufs=1))
    w1s = ctx.enter_context(tc.tile_pool(name="w1s", bufs=2))
    hwp = ctx.enter_context(tc.tile_pool(name="hwp", bufs=2))
    hp = ctx.enter_context(tc.tile_pool(name="hp", bufs=1))
    hq = ctx.enter_context(tc.tile_pool(name="hq", bufs=1))
    qkp = ctx.enter_context(tc.tile_pool(name="qkp", bufs=1))
    scr = ctx.enter_context(tc.tile_pool(name="scr", bufs=2))
    expp = ctx.enter_context(tc.tile_pool(name="expp", bufs=8))
    otp = ctx.enter_context(tc.tile_pool(name="otp", bufs=1))
    agf = ctx.enter_context(tc.tile_pool(name="agf", bufs=2))   # AG reads
    d8 = ctx.enter_context(tc.tile_pool(name="d8", bufs=1))     # attn delta
    dfp = ctx.enter_context(tc.tile_pool(name="dfp", bufs=1))   # RS-out quarter
    uqp = ctx.enter_context(tc.tile_pool(name="uqp", bufs=1))   # mlp hidden q
    lgp = ctx.enter_context(tc.tile_pool(name="lgp", bufs=2))
    tiny = ctx.enter_context(tc.tile_pool(name="tiny", bufs=2))
    rows1 = ctx.enter_context(tc.tile_pool(name="rows1", bufs=1))
    rows3 = ctx.enter_context(tc.tile_pool(name="rows3", bufs=2))
    rows2 = ctx.enter_context(tc.tile_pool(name="rows2", bufs=1))
    bcp = ctx.enter_context(tc.tile_pool(name="bcp", bufs=1))
    rbp = ctx.enter_context(tc.tile_pool(name="rbp", bufs=1))
    psmm = ctx.enter_context(tc.tile_pool(name="psmm", bufs=4, space="PSUM"))
    psaux = ctx.enter_context(tc.tile_pool(name="psaux", bufs=3, space="PSUM"))
    psst = ctx.enter_context(tc.tile_pool(name="psst", bufs=1, space="PSUM"))
    dram = ctx.enter_context(tc.tile_pool(name="dram", bufs=1, space="DRAM"))

    # ---------------- constants ----------------
    ones_col = sing.tile([128, 1], BF, name="ones_col")
    nc.vector.memset(ones_col, 1.0)
    ones_row = sing.tile([1, 128], BF, name="ones_row")
    nc.vector.memset(ones_row, 1.0)
    ones_row_f = sing.tile([1, 128], F32, name="ones_row_f")
    nc.vector.memset(ones_row_f, 1.0)
    eps_ap = sing.tile([1, 1], F32, name="eps_ap")
    nc.vector.memset(eps_ap, EPS)
    sel2 = sing.tile([65, 128], F32, name="sel2")
    nc.sync.dma_start(out=sel2, in_=v["sel2d"][:, :])
    # causal diagonal-band masks, one per key-block offset (built once)
    maskt = sing.tile([128, 4, TCH], BF, name="maskt")
    nc.vector.memset(maskt, 1.0)
    for o in range(4):
        nc.gpsimd.affine_select(
            out=maskt[:, o, :], in_=maskt[:, o, :], pattern=[[1, TCH]],
            compare_op=AL.is_ge, fill=0.0, base=-(o * 128),
            channel_multiplier=-1)

    # ---------------- residual stream ----------------
    TQ = T // TP                 # local MLP tokens per rank (256)
    TQB = TQ // NCH              # per token-half block (128)
    x = [sing.tile([128, T], F32, name=f"x{k}") for k in range(KT)]
    xq = [sing.tile([128, TQ], F32, name=f"xq{k}") for k in range(KT)]
    for k in range(KT):
        nc.sync.dma_start(out=x[k], in_=xT0[k * 128:(k + 1) * 128, :])
        nc.sync.dma_start(out=xq[k], in_=v["xq0"][k * 128:(k + 1) * 128, :])

    # ---------------- layernorm ----------------
    def layernorm(x_tiles, grow_dram, brow_dram, name, ncols=None, nch=None,
                  hpool=None, htag="h"):
        """LN over the feature (partition) axis of transposed activations.
        Chunk-outer so chunk 0 proceeds while chunk 1's inputs are still
        being gathered. Returns bf16 tiles h[kt] = LN(x)."""
        ncols = T if ncols is None else ncols
        nch = NCH if nch is None else nch
        hpool = hp if hpool is None else hpool
        tchl = ncols // nch
        # per-partition gamma/beta columns: [128, KT]
        gcol = tiny.tile([128, KT], F32, name=f"g_{name}")
        bcol = tiny.tile([128, KT], F32, name=f"b_{name}")
        nc.scalar.dma_start(out=gcol, in_=grow_dram.rearrange("(k p) -> p k", p=128))
        nc.scalar.dma_start(out=bcol, in_=brow_dram.rearrange("(k p) -> p k", p=128))

        h = [hpool.tile([128, ncols], BF, name=f"h_{name}_{k}",
                        tag=f"{htag}{k}")
             for k in range(KT)]
        for ch in range(nch):
            cs = slice(ch * tchl, (ch + 1) * tchl)
            # stats: PSUM tile holds sum at partition 0, sumsq at 32.
            ps_st = psst.tile([33, tchl], F32, name="ps_st", tag="ps_st")
            for k in range(KT):
                xbt = scr.tile([128, tchl], BF, name="xb", tag="xb")
                nc.vector.tensor_copy(xbt, x_tiles[k][:, cs])
                sqt = scr.tile([128, tchl], BF, name="sq", tag="sq")
                nc.scalar.square(sqt, xbt)
                nc.tensor.matmul(ps_st[0:1, :], ones_col, xbt,
                                 start=(k == 0), stop=(k == KT - 1))
                nc.tensor.matmul(ps_st[32:33, :], ones_col, sqt,
                                 start=(k == 0), stop=(k == KT - 1))
            # moments for this chunk
            st_sb = rows1.tile([1, 2 * tchl], F32, name=f"st_{name}", tag="st_sb")
            nc.vector.tensor_copy(st_sb[:, 0:tchl], ps_st[0:1, :])
            nc.vector.tensor_copy(st_sb[:, tchl:2 * tchl], ps_st[32:33, :])
            mom = rows1.tile([1, 2 * tchl], F32, name=f"mom_{name}", tag="mom")
            nc.scalar.mul(mom, st_sb, 1.0 / D)      # [mean | E[x^2]]
            mean = mom[:, 0:tchl]
            msq = mom[:, tchl:2 * tchl]
            m2 = rows3.tile([1, tchl], F32, name=f"m2_{name}", tag="row1k")
            nc.vector.tensor_mul(m2, mean, mean)
            var = rows3.tile([1, tchl], F32, name=f"var_{name}", tag="row1k")
            nc.vector.tensor_tensor(out=var, in0=msq, in1=m2, op=AL.subtract)
            sd = rows3.tile([1, tchl], F32, name=f"sd_{name}", tag="row1k")
            nc.scalar.activation(sd, var, AF.Sqrt, bias=eps_ap)
            rstd = rows3.tile([1, tchl], F32, name=f"rstd_{name}", tag="row1k")
            nc.vector.reciprocal(rstd, sd)
            nmr = rows3.tile([1, tchl], F32, name=f"nmr_{name}", tag="row1k")
            nc.vector.tensor_mul(nmr, mean, rstd)
            nc.scalar.mul(nmr, nmr, -1.0)           # -mean*rstd
            # broadcast to [128, tchl] via K=1 outer-product matmuls (fp32)
            rstdB = bcp.tile([128, tchl], F32, name="rstdB", tag="rstdB")
            nmB = bcp.tile([128, tchl], F32, name="nmB", tag="nmB")
            pb = psaux.tile([128, tchl], F32, name="pb", tag="aux")
            nc.tensor.matmul(pb, ones_row_f, rstd, start=True, stop=True)
            nc.scalar.copy(rstdB, pb)
            pb2 = psaux.tile([128, tchl], F32, name="pb2", tag="aux")
            nc.tensor.matmul(pb2, ones_row_f, nmr, start=True, stop=True)
            nc.scalar.copy(nmB, pb2)
            # apply: h = (x*rstdB + nmB)*g + b, output bf16
            for k in range(KT):
                t1 = scr.tile([128, tchl], BF, name="lnt", tag="lnt")
                nc.vector.tensor_mul(t1, x_tiles[k][:, cs], rstdB)
                t2 = scr.tile([128, tchl], BF, name="lnt2", tag="lnt2")
                nc.vector.tensor_tensor(out=t2, in0=t1, in1=nmB, op=AL.add)
                nc.vector.tensor_scalar(
                    out=h[k][:, cs], in0=t2, scalar1=gcol[:, k:k + 1],
                    scalar2=bcol[:, k:k + 1], op0=AL.mult, op1=AL.add)
        return h

    # ---------------- transformer layers ----------------
    for l in range(L):
        # -- weights for this layer --
        wqt = wts.tile([128, KT, DSH], BF, name="wqt", tag="wqt")
        wkt = wts.tile([128, KT, DSH], BF, name="wkt", tag="wkt")
        wvt = wts.tile([128, KT, DSH], BF, name="wvt", tag="wvt")
        for dst, srcw in ((wqt, wq), (wkt, wk), (wvt, wv)):
            nc.scalar.dma_start(out=dst, in_=srcw[l])
        # row-sharded Wo: [DSH local head feats, D] -> [128, 2, D]
        wot = wts.tile([128, MSH, D], BF, name="wot", tag="wot")
        nc.scalar.dma_start(out=wot, in_=wo[l])
        b1col = tiny.tile([128, KTF], F32, name="b1col", tag="b1col")
        nc.scalar.dma_start(out=b1col, in_=b1d[l].rearrange("(k p) -> p k", p=128))
        b2col = tiny.tile([128, KT], F32, name="b2col", tag="b2col")
        nc.scalar.dma_start(out=b2col, in_=b2d[l].rearrange("(k p) -> p k", p=128))

        # -- LN1 --
        h1 = layernorm(x, g1d[l], be1d[l], f"ln1_{l}")

        # -- QKV projections (chunk-outer so attention c0 starts early) --
        # qT/kT: [DSH, T] transposed; v: natural [T, DSH] + ones column
        qT = [qkp.tile([128, T], BF, name=f"qT{m}", tag=f"qT{m}")
              for m in range(MSH)]
        kTt = [qkp.tile([128, T], BF, name=f"kT{m}", tag=f"kT{m}")
               for m in range(MSH)]
        vt = qkp.tile([128, TKT, HL, DK + 1], BF, name="vt", tag="vt")
        nc.vector.memset(vt[:, :, :, DK:DK + 1], 1.0)
        for chn in range(NCH):
            cs = slice(chn * TCH, (chn + 1) * TCH)
            for wt, dst in ((wkt, kTt), (wqt, qT)):
                pq = {}
                for m in range(MSH):
                    pq[m] = psmm.tile([128, TCH], F32, name="ps", tag="mm")
                for k in range(KT):
                    for m in range(MSH):
                        nc.tensor.matmul(pq[m],
                                         wt[:, k, m * 128:(m + 1) * 128],
                                         h1[k][:, cs],
                                         start=(k == 0), stop=(k == KT - 1))
                for m in range(MSH):
                    nc.vector.tensor_copy(dst[m][:, cs], pq[m])
            for t in range(chn * (TCH // 128), (chn + 1) * (TCH // 128)):
                ps = psmm.tile([128, TCH], F32, name="psv", tag="mm")
                for k in range(KT):
                    nc.tensor.matmul(ps[:, 0:DSH],
                                     h1[k][:, t * 128:(t + 1) * 128],
                                     wvt[:, k, :],
                                     start=(k == 0), stop=(k == KT - 1))
                nc.vector.tensor_copy(
                    vt[:, t, :, 0:DK],
                    ps[:, 0:DSH].rearrange("p (h d) -> p h d", h=HL))

        # -- attention, chunk-outer; Wo row-shard -> token ReduceScatter --
        rs_in = [dram.tile([TP, D, TQB], BF, name=f"rs_in{l}_{c}")
                 for c in range(NCH)]
        rs_out = [dram.tile([D, TQB], BF, name=f"rs_out{l}_{c}")
                  for c in range(NCH)]
        for chn in range(NCH):
            cs = slice(chn * TCH, (chn + 1) * TCH)
            jmax = (chn + 1) * (TCH // 128)
            oT = [otp.tile([128, TCH], BF, name=f"oT{m}", tag=f"oT{m}")
                  for m in range(MSH)]
            for hpi in range(HL // 2):
                # heads (2hp, 2hp+1) sit at partitions 0-63 / 64-127 of
                # m-tile hp: their K=64 score matmuls land in different PE
                # row-groups and run concurrently.
                mt = hpi
                ps_os = [psaux.tile([DK + 1, TCH], F32, name="ps_o",
                                    tag="aux") for _ in range(2)]
                for j in range(jmax):
                    ets = []
                    for sub in range(2):
                        po = sub * DK
                        pss = psmm.tile([128, TCH], F32, name="pss",
                                        tag="mm")
                        nc.tensor.matmul(
                            pss, kTt[mt][po:po + DK, j * 128:(j + 1) * 128],
                            qT[mt][po:po + DK, cs], start=True, stop=True)
                        et = expp.tile([128, TCH], BF, name="exp", tag="exp")
                        nc.scalar.activation(et, pss, AF.Exp, scale=0.125)
                        if j * 128 >= chn * TCH:
                            # zero where tk_global > tq_global
                            nc.vector.tensor_mul(
                                et, et, maskt[:, j - chn * 4, :])
                        ets.append(et)
                    for sub in range(2):
                        nc.tensor.matmul(ps_os[sub], vt[:, j, 2 * hpi + sub, :],
                                         ets[sub],
                                         start=(j == 0), stop=(j == jmax - 1))
                den2 = rows2.tile([65, TCH], F32, name="den2", tag="den")
                nc.vector.memset(den2, 1.0)
                nc.vector.tensor_copy(den2[0:1, :], ps_os[0][DK:DK + 1, :])
                nc.vector.tensor_copy(den2[DK:DK + 1, :],
                                      ps_os[1][DK:DK + 1, :])
                rec2 = rows2.tile([65, TCH], F32, name="rec2", tag="rec")
                rsc2 = rows2.tile([65, TCH], F32, name="rsc2", tag="rsc")
                nc.vector.reciprocal_approx_accurate(rec2, den2, rsc2)
                ps_r = psmm.tile([128, TCH], F32, name="ps_r", tag="mm")
                nc.tensor.matmul(ps_r, sel2, rec2, start=True, stop=True)
                rb = rbp.tile([128, TCH], BF, name="rb", tag="rb")
                nc.vector.tensor_copy(rb, ps_r)
                for sub in range(2):
                    po = sub * DK
                    nc.vector.tensor_tensor(
                        out=oT[mt][po:po + DK, :], in0=ps_os[sub][0:DK, :],
                        in1=rb[po:po + DK, :], op=AL.mult)
            # Wo row-shard: d1_part[m] = Wo[local rows].T @ o_local, full D
            d1_sb = [d8.tile([128, TCH], BF, name=f"d1s{m}", tag=f"d1s{m}")
                     for m in range(KT)]
            for m in range(KT):
                psd = psmm.tile([128, TCH], F32, name="psd1", tag="mm")
                for kk in range(MSH):
                    nc.tensor.matmul(psd,
                                     wot[:, kk, m * 128:(m + 1) * 128],
                                     oT[kk],
                                     start=(kk == 0), stop=(kk == MSH - 1))
                nc.vector.tensor_copy(d1_sb[m], psd)
                for b in range(TCH // TQB):
                    nc.sync.dma_start(
                        out=rs_in[chn][b, m * 128:(m + 1) * 128, :],
                        in_=d1_sb[m][:, b * TQB:(b + 1) * TQB])
            nc.gpsimd.collective_compute(
                "ReduceScatter", AL.add, replica_groups=groups,
                ins=[rs_in[chn].opt()], outs=[rs_out[chn].opt()])

        # -- local token-quarter residual: xq += d1q (kept for delta AG) --
        dfq = [dfp.tile([128, TQ], BF, name=f"dfq{k}", tag=f"dfq{k}")
               for k in range(KT)]
        for c in range(NCH):
            qs = slice(c * TQB, (c + 1) * TQB)
            for k in range(KT):
                nc.sync.dma_start(out=dfq[k][:, qs],
                                  in_=rs_out[c][k * 128:(k + 1) * 128, :])
                nc.vector.tensor_tensor(out=xq[k][:, qs], in0=xq[k][:, qs],
                                        in1=dfq[k][:, qs], op=AL.add)

        # -- LN2 on local quarter + full-width MLP on 256 tokens --
        h2q = layernorm(xq, g2d[l], be2d[l], f"ln2_{l}", ncols=TQ, nch=NCH,
                        hpool=hq, htag="hq")
        uq = uqp.tile([128, KTF, TQ], BF, name="uq", tag="uq")
        for m in range(KTF):
            w1m = w1s.tile([128, KT, 128], BF, name="w1m", tag="w1m")
            nc.scalar.dma_start(out=w1m, in_=w1[l][:, m])
            pu = psmm.tile([128, TQ], F32, name="psu", tag="mm")
            for k in range(KT):
                nc.tensor.matmul(pu, w1m[:, k, :], h2q[k],
                                 start=(k == 0), stop=(k == KT - 1))
            nc.scalar.activation(uq[:, m, :], pu, AF.Gelu,
                                 bias=b1col[:, m:m + 1])

        # -- W2 full-width on local tokens; delta = d1q + z + b2 --
        for m in range(KT):
            w2m = w1s.tile([128, KTF, 128], BF, name="w2m", tag="w2m")
            nc.scalar.dma_start(out=w2m, in_=w2[l][:, m])
            psz = psmm.tile([128, TQ], F32, name="psz", tag="mm")
            for kk in range(KTF):
                nc.tensor.matmul(psz, w2m[:, kk, :], uq[:, kk, :],
                                 start=(kk == 0), stop=(kk == KTF - 1))
            nc.vector.scalar_tensor_tensor(
                out=xq[m], in0=psz, scalar=b2col[:, m:m + 1],
                in1=xq[m], op0=AL.add, op1=AL.add)
            nc.vector.scalar_tensor_tensor(
                out=dfq[m], in0=psz, scalar=b2col[:, m:m + 1],
                in1=dfq[m], op0=AL.add, op1=AL.add)

        # -- AllGather layer deltas per token half; update full x --
        ag_in = [dram.tile([D, TQB], BF, name=f"ag_in{l}_{c}")
                 for c in range(NCH)]
        ag_out = [dram.tile([TP, D, TQB], BF, name=f"ag_out{l}_{c}")
                  for c in range(NCH)]
        for c in range(NCH):
            qs = slice(c * TQB, (c + 1) * TQB)
            for k in range(KT):
                nc.sync.dma_start(out=ag_in[c][k * 128:(k + 1) * 128, :],
                                  in_=dfq[k][:, qs])
            nc.gpsimd.collective_compute(
                "AllGather", AL.bypass, replica_groups=groups,
                ins=[ag_in[c].opt()], outs=[ag_out[c].opt()])
        for c in range(NCH):
            for rr in range(TP):
                xf = agf.tile([128, KT, TQB], BF, name="xf", tag="agf")
                nc.sync.dma_start(
                    out=xf,
                    in_=ag_out[c][rr].rearrange("(k p) t -> p k t", p=128))
                tb = c * TCH + rr * TQB
                for k in range(KT):
                    nc.vector.tensor_tensor(
                        out=x[k][:, tb:tb + TQB], in0=x[k][:, tb:tb + TQB],
                        in1=xf[:, k, :], op=AL.add)

    # ---------------- final LN + logits ----------------
    hf = layernorm(x, gfd[0], befd[0], "lnf")
    for n in range(NV):
        hb = hwp.tile([128, KT, VCH], BF, name="hwb", tag="hwb")
        nc.scalar.dma_start(out=hb, in_=hwd[n])
        for t in range(TT):
            ps = psmm.tile([128, TCH], F32, name="pslg", tag="mm")
            for k in range(KT):
                nc.tensor.matmul(ps[:, 0:VCH],
                                 hf[k][:, t * 128:(t + 1) * 128],
                                 hb[:, k, :],
                                 start=(k == 0), stop=(k == KT - 1))
            lg = lgp.tile([128, VCH], F32, name="lg", tag="lg")
            nc.vector.tensor_copy(lg, ps[:, 0:VCH])
            nc.sync.dma_start(
                out=logits[t * 128:(t + 1) * 128, n * VCH:(n + 1) * VCH],
                in_=lg)

    ctx.close()


# ---------------- host side ----------------

_PROG_CACHE = {}


def _get_program():
    if "nc" not in _PROG_CACHE:
        _PROG_CACHE["nc"] = build_program()
    return _PROG_CACHE["nc"]


def make_in_maps(input_ids, emb, Wq, Wk, Wv, Wo, W1, b1, W2, b2,
                 ln1_g, ln1_b, ln2_g, ln2_b, lnf_g, lnf_b, head_w):
    TP = CFG["TP"]
    D, V = CFG["D"], CFG["V"]
    DSH, DFS, VSH = D // TP, 4 * D // TP, V // TP
    bf = ml_dtypes.bfloat16
    in_maps = []
    S = CFG["S"]
    L = CFG["L"]
    TQB = S // TP // 2
    KT, KTF = D // 128, 4 * D // 128
    w1f = np.ascontiguousarray(
        np.asarray(W1).reshape(L, KT, 128, KTF, 128)
        .transpose(0, 2, 3, 1, 4)).astype(bf)
    w2f = np.ascontiguousarray(
        np.asarray(W2).reshape(L, KTF, 128, KT, 128)
        .transpose(0, 2, 3, 1, 4)).astype(bf)
    b1f = np.ascontiguousarray(b1).astype(np.float32)
    def _wtile(wfull):
        # [L, D, DSH] -> [L, 128, KT, DSH]
        a = np.asarray(wfull)
        return np.ascontiguousarray(
            a.reshape(L, KT, 128, a.shape[-1]).transpose(0, 2, 1, 3))
    for c in range(N_CORES):
        g, r = c // TP, c % TP
        x0 = np.asarray(emb)[np.asarray(input_ids)[g]]          # [S, D] f32
        x0T = np.ascontiguousarray(x0.T).astype(np.float32)
        xq0 = np.concatenate(
            [x0T[:, r * TQB:(r + 1) * TQB],
             x0T[:, S // 2 + r * TQB:S // 2 + (r + 1) * TQB]], axis=1)
        sel2 = np.zeros((65, 128), np.float32)
        sel2[0, 0:64] = 1.0
        sel2[64, 64:128] = 1.0
        in_maps.append({
            "sel2d": sel2,
            "xT0": x0T,
            "xq0": np.ascontiguousarray(xq0).astype(np.float32),
            "wq": _wtile(Wq[:, :, r * DSH:(r + 1) * DSH]).astype(bf),
            "wk": _wtile(Wk[:, :, r * DSH:(r + 1) * DSH]).astype(bf),
            "wv": _wtile(Wv[:, :, r * DSH:(r + 1) * DSH]).astype(bf),
            "wo": np.ascontiguousarray(
                np.asarray(Wo)[:, r * DSH:(r + 1) * DSH, :]
                .reshape(L, DSH // 128, 128, D)
                .transpose(0, 2, 1, 3)).astype(bf),
            "w1": w1f,
            "w2": w2f,
            "b1": b1f,
            "b2": np.asarray(b2, dtype=np.float32),
            "g1": np.asarray(ln1_g, dtype=np.float32),
            "be1": np.asarray(ln1_b, dtype=np.float32),
            "g2": np.asarray(ln2_g, dtype=np.float32),
            "be2": np.asarray(ln2_b, dtype=np.float32),
            "gf": np.asarray(lnf_g, dtype=np.float32).reshape(1, -1),
            "bef": np.asarray(lnf_b, dtype=np.float32).reshape(1, -1),
            "hw": np.ascontiguousarray(
                np.asarray(head_w)[:, r * VSH:(r + 1) * VSH]
                .reshape(KT, 128, 16, 500).transpose(2, 1, 0, 3)).astype(bf),
        })
    return in_maps


def kernel(**inputs):
    B, S, V = CFG["B"], CFG["S"], CFG["V"]
    TP = CFG["TP"]
    VSH = V // TP
    nc = _get_program()
    in_maps = make_in_maps(**inputs)
    res = run_bass_kernel_spmd(nc, in_maps, list(range(N_CORES)), trace=False)
    out = np.empty((B, S, V), dtype=np.float32)
    for c in range(N_CORES):
        g, r = c // TP, c % TP
        out[g, :, r * VSH:(r + 1) * VSH] = res.results[c]["logits"]
    return out


def run_traced(**inputs):
    """Like kernel() but with NTFF tracing; returns (out, exec_time_ns)."""
    nc = _get_program()
    in_maps = make_in_maps(**inputs)
    res = run_bass_kernel_spmd(nc, in_maps, list(range(N_CORES)), trace=True)
    B, S, V = CFG["B"], CFG["S"], CFG["V"]
    TP = CFG["TP"]
    VSH = V // TP
    out = np.empty((B, S, V), dtype=np.float32)
    for c in range(N_CORES):
        g, r = c // TP, c % TP
        out[g, :, r * VSH:(r + 1) * VSH] = res.results[c]["logits"]
    return out, res.exec_time_ns

